# revision 21
# baseline (speedup 1.0000x reference)
"""GCN layers (3x GCNConv + PReLU + residual + BatchNorm) on 8 TRN2 NeuronCores.

Full-input contract: kernel(**inputs) takes unsharded numpy arrays and returns
the full [50000, 64] float32 output.

Sharding (graph/data parallel): nodes are partitioned into 8 contiguous
ranges; edges are bucketed to the core owning their dst node and grouped by
dst block. Per layer, per core:
  1. xw^T = W[i]^T @ h^T  (PE, feature-major), transposed+converted into a
     bf16 node-major gather table shard (padded rows of 128 cols, 64 valid)
  2. Two AllGathers build two table pieces in local HBM (the split keeps
     gather indices within int16 range, and lets table-A gathers overlap
     the second collective)
  3. stream edges: dma_gather 256B rows table[src] -> SBUF; aggregate with
     per-128-edge-subchunk segment matmuls agg^T[block] += msg^T @ S^T
     (S carries the edge weights, bf16, PSUM-accumulated per dst block)
  4. epilogue (feature-major [64, nodes]): +bias, PReLU, residual, BatchNorm
     with global batch stats via a tiny AllReduce; per-feature params are
     per-partition scalars (native tensor_scalar ops).

The per-block subchunk schedule is made uniform across cores (max-padded)
so all cores run the same SPMD program.
"""

import os
import numpy as np

N_NODES = 50000
D = 64
L = 3
BN_EPS = 1e-5
N_CORES = 8
GCHUNK = 6144           # edge slots per gather chunk (48 subchunks of 128)
BLKN = 256              # dst nodes per aggregation block (S columns)
IDX_LIMIT = 32768       # int16 gather index range

LAST_RUN = {}


# ----------------------------------------------------------------------------
# Host-side preprocessing
# ----------------------------------------------------------------------------

def _wrap16(flat, slots):
    """Edge-slot array -> [128, slots/16] int16 'wrapped' index layout."""
    a = flat.reshape(slots // 16, 16).T.astype(np.int16)
    return np.tile(a, (8, 1))


def _preprocess(x, edge_src, edge_dst, edge_weight, W, b, prelu_a,
                bn_gamma, bn_beta, n_cores, nsh, gchunk, blkn,
                split_ag=True):
    import ml_dtypes
    bf16 = ml_dtypes.bfloat16

    n = x.shape[0]
    d = x.shape[1]
    nt = (nsh + 127) // 128
    npad = nt * 128
    subc = gchunk // 128
    nblk = (npad + blkn - 1) // blkn
    # per-rank local-row split: table A = local rows [0, asplit), B = rest
    if npad * n_cores <= IDX_LIMIT:
        asplit = max(128, (npad // 2) // 128 * 128)
    else:
        asplit = (IDX_LIMIT // n_cores) // 128 * 128

    src = np.asarray(edge_src).astype(np.int64)
    dst = np.asarray(edge_dst).astype(np.int64)
    w = np.asarray(edge_weight).astype(np.float32)
    x = np.asarray(x).astype(np.float32)

    s_rank = src // nsh
    s_loc = src % nsh
    if split_ag:
        inA = s_loc < asplit
        idxA = s_rank * asplit + s_loc
        idxB = s_rank * (npad - asplit) + (s_loc - asplit)
    else:
        src_pad = s_rank * npad + s_loc
        gsplit = min(IDX_LIMIT, ((npad * n_cores) // 2) // 128 * 128)
        inA = src_pad < gsplit
        idxA = src_pad
        idxB = src_pad - gsplit
        asplit = gsplit  # reused as the global row split below
    shard = dst // nsh
    dst_local = dst % nsh

    streams = []
    for sel, tix in ((inA, idxA), (~inA, idxB)):
        per_core_edges = []
        cnts = []
        for r in range(n_cores):
            m = (shard == r) & sel
            per_core_edges.append((tix[m], dst_local[m], w[m]))
            cnts.append(np.bincount(dst_local[m] // blkn, minlength=nblk))
        nsub = np.zeros(nblk, np.int64)
        for c in cnts:
            nsub = np.maximum(nsub, (c + 127) // 128)
        sub_off = np.concatenate([[0], np.cumsum(nsub)])
        total_subs = int(sub_off[-1])
        nch = max(1, (total_subs + subc - 1) // subc)
        padded_subs = nch * subc
        slots = padded_subs * 128

        sched = []
        for blk in range(nblk):
            for j in range(int(nsub[blk])):
                gsub = int(sub_off[blk]) + j
                sched.append((gsub // subc, gsub % subc, blk,
                              j == 0, j == int(nsub[blk]) - 1))

        per_core = []
        for r in range(n_cores):
            ti, dl, wr = per_core_edges[r]
            blk = dl // blkn
            col = dl % blkn
            order = np.argsort(blk, kind="stable")
            ti, wr, blk, col = (a[order] for a in (ti, wr, blk, col))
            cnt = cnts[r]
            starts = np.concatenate([[0], np.cumsum(cnt)])
            pos = np.arange(len(ti)) - starts[blk]
            gsub = sub_off[blk] + pos // 128
            row = pos % 128
            slot = gsub * 128 + row
            rng_pad = np.random.default_rng(12345 + r)
            trows_s = int(ti.max()) + 1 if len(ti) else 1
            idx = rng_pad.integers(0, trows_s, slots)
            idx[slot] = ti
            S = np.zeros((padded_subs, 128, blkn), np.float32)
            S[gsub, row, col] = wr
            St = np.ascontiguousarray(S.transpose(1, 0, 2)
                                      .reshape(128, padded_subs * blkn))
            per_core.append((_wrap16(idx, slots), St.astype(bf16)))
        streams.append(dict(nch=nch, slots=slots, sched=sched,
                            per_core=per_core))

    bT = np.ascontiguousarray(np.asarray(b, np.float32).T)
    gammaT = np.ascontiguousarray(np.asarray(bn_gamma, np.float32).T)
    betaT = np.ascontiguousarray(np.asarray(bn_beta, np.float32).T)
    prelu_rep = np.tile(np.asarray(prelu_a, np.float32).reshape(1, L),
                        (128, 1))
    Wf = np.ascontiguousarray(np.asarray(W, np.float32))

    in_maps = []
    for r in range(n_cores):
        h0T = np.zeros((d, npad), np.float32)
        h0T[:, :nsh] = x[r * nsh:(r + 1) * nsh].T
        in_maps.append({
            "h0sT": h0T,
            "Wp": Wf,
            "bT": bT,
            "gammaT": gammaT,
            "betaT": betaT,
            "prelu_rep": prelu_rep,
            "srcA": streams[0]["per_core"][r][0],
            "Sa": streams[0]["per_core"][r][1],
            "srcB": streams[1]["per_core"][r][0],
            "Sb": streams[1]["per_core"][r][1],
        })

    cfg = dict(n_cores=n_cores, nsh=nsh, d=d, nt=nt, npad=npad,
               gchunk=gchunk, subc=subc, blkn=blkn, nblk=nblk,
               asplit=asplit, split_ag=split_ag, n_nodes=n,
               nchA=streams[0]["nch"], slotsA=streams[0]["slots"],
               schedA=streams[0]["sched"],
               nchB=streams[1]["nch"], slotsB=streams[1]["slots"],
               schedB=streams[1]["sched"])
    return in_maps, cfg


# ----------------------------------------------------------------------------
# Device program
# ----------------------------------------------------------------------------

def _build_nc(cfg):
    import concourse.bacc as bacc
    import concourse.tile as tile
    import concourse.mybir as mybir
    from concourse import library_config
    from concourse.masks import make_identity

    fp32 = mybir.dt.float32
    bf16 = mybir.dt.bfloat16
    i16 = mybir.dt.int16
    Alu = mybir.AluOpType
    Ax = mybir.AxisListType

    n_cores = cfg["n_cores"]
    nsh, d, nt, npad = cfg["nsh"], cfg["d"], cfg["nt"], cfg["npad"]
    gchunk, subc = cfg["gchunk"], cfg["subc"]
    blkn, nblk, asplit = cfg["blkn"], cfg["nblk"], cfg["asplit"]
    n_nodes = cfg["n_nodes"]
    slotsA, slotsB = cfg["slotsA"], cfg["slotsB"]
    nchA, nchB = cfg["nchA"], cfg["nchB"]
    split_ag = cfg.get("split_ag", True)
    if split_ag:
        rows_a = n_cores * asplit
        rows_b = n_cores * (npad - asplit)
    else:
        rows_a = asplit
        rows_b = n_cores * npad - asplit
    i16s = gchunk // 16
    nxc = (npad + 511) // 512

    def by_chunk(sched, nch):
        per = [[] for _ in range(nch)]
        for (c, j, blk, st, sp) in sched:
            per[c].append((j, blk, st, sp))
        return per

    schedA = by_chunk(cfg["schedA"], nchA)
    schedB = by_chunk(cfg["schedB"], nchB)
    skip_gather = cfg.get("skip_gather", False)
    skip_smm = cfg.get("skip_smm", False)
    skip_edges = cfg.get("skip_edges", False)
    skip_ag = cfg.get("skip_ag", False)

    nc = bacc.Bacc(None, target_bir_lowering=False, debug=False)

    h0sT = nc.declare_dram_parameter("h0sT", [d, npad], fp32, isOutput=False)
    Wp = nc.declare_dram_parameter("Wp", [L, d, d], fp32, isOutput=False)
    bT_in = nc.declare_dram_parameter("bT", [d, L], fp32, isOutput=False)
    gammaT_in = nc.declare_dram_parameter("gammaT", [d, L], fp32, isOutput=False)
    betaT_in = nc.declare_dram_parameter("betaT", [d, L], fp32, isOutput=False)
    prelu_in = nc.declare_dram_parameter("prelu_rep", [128, L], fp32, isOutput=False)
    srcA = nc.declare_dram_parameter("srcA", [128, slotsA // 16], i16, isOutput=False)
    Sa = nc.declare_dram_parameter("Sa", [128, slotsA * blkn // 128], bf16, isOutput=False)
    srcB = nc.declare_dram_parameter("srcB", [128, slotsB // 16], i16, isOutput=False)
    Sb = nc.declare_dram_parameter("Sb", [128, slotsB * blkn // 128], bf16, isOutput=False)
    out_ext = nc.declare_dram_parameter("out", [npad, d], fp32, isOutput=True)

    with tile.TileContext(nc) as tc:
        with (
            tc.tile_pool(name="const", bufs=1) as cpool,
            tc.tile_pool(name="state", bufs=1) as spool,
            tc.tile_pool(name="meta", bufs=1) as epool,
            tc.tile_pool(name="work", bufs=2) as wpool,
            tc.tile_pool(name="rows", bufs=2) as rpool,
            tc.tile_pool(name="msg", bufs=2) as mpool,
            tc.tile_pool(name="smat", bufs=2) as stpool,
            tc.tile_pool(name="ps", bufs=2, space="PSUM") as ppool,
            tc.tile_pool(name="psagg", bufs=2, space="PSUM") as apool,
            tc.tile_pool(name="dram", bufs=1, space="DRAM") as dpool,
        ):
            h_sb = spool.tile([d, npad], fp32, tag="h")
            p_sb = spool.tile([d, npad], fp32, tag="p")
            t_sb = spool.tile([d, npad], fp32, tag="t")
            xwbf = spool.tile([128, nt, 128], bf16, tag="xwbf")
            out_sb = spool.tile([128, nt, d], fp32, tag="osb")
            ident = cpool.tile([d, d], fp32, tag="ident")
            W_sb = cpool.tile([d, L * d], fp32, tag="Wsb")
            bT_sb = cpool.tile([d, L], fp32, tag="bT")
            gaT_sb = cpool.tile([d, L], fp32, tag="gaT")
            beT_sb = cpool.tile([d, L], fp32, tag="beT")
            prelu_sb = cpool.tile([128, L], fp32, tag="prelu")
            stat_sb = spool.tile([d, 2], fp32, tag="stat")
            stat2_sb = spool.tile([d, 2], fp32, tag="stat2")
            iA_sb = epool.tile([128, slotsA // 16], i16, tag="iA")
            iB_sb = epool.tile([128, slotsB // 16], i16, tag="iB")

            if split_ag:
                xw_bounceA = dpool.tile([asplit, 128], bf16, tag="xwbA")
                xw_bounceB = dpool.tile([npad - asplit, 128], bf16, tag="xwbB")
                xw_fullA = dpool.tile([rows_a, 128], bf16, tag="xwfA")
                xw_fullB = dpool.tile([rows_b, 128], bf16, tag="xwfB")
                tblA_ap, tblB_ap = xw_fullA[:], xw_fullB[:]
            else:
                xw_bounce1 = dpool.tile([npad, 128], bf16, tag="xwb1")
                xw_full1 = dpool.tile([n_cores * npad, 128], bf16, tag="xwf1")
                tblA_ap = xw_full1[0:rows_a, :]
                tblB_ap = xw_full1[rows_a:n_cores * npad, :]
            stats_in = dpool.tile([2, d], fp32, tag="sin")
            stats_out = dpool.tile([2, d], fp32, tag="sout")

            nc.sync.dma_start(h_sb[:], h0sT[:])
            for i in range(L):
                nc.sync.dma_start(W_sb[:, i * d:(i + 1) * d], Wp[i, :, :])
            nc.sync.dma_start(bT_sb[:], bT_in[:])
            nc.sync.dma_start(gaT_sb[:], gammaT_in[:])
            nc.sync.dma_start(beT_sb[:], betaT_in[:])
            nc.sync.dma_start(prelu_sb[:], prelu_in[:])
            nc.sync.dma_start(iA_sb[:], srcA[:])
            nc.sync.dma_start(iB_sb[:], srcB[:])
            make_identity(nc, ident[:])
            nc.vector.memset(xwbf[:], 0.0)
            nc.gpsimd.load_library(library_config.mlp)

            for i in range(L):
                # ---- xw^T = W[i]^T @ h^T, transpose+convert -----------------
                for c in range(nxc):
                    lo = c * 512
                    hi = min(npad, lo + 512)
                    xw_ps = ppool.tile([d, 512], fp32, tag="xwps")
                    nc.tensor.matmul(xw_ps[:, :hi - lo],
                                     lhsT=W_sb[:, i * d:(i + 1) * d],
                                     rhs=h_sb[:, lo:hi], start=True, stop=True)
                    xwT_tmp = wpool.tile([d, 512], fp32, tag="xwt")
                    nc.vector.tensor_copy(xwT_tmp[:, :hi - lo],
                                          xw_ps[:, :hi - lo])
                    for k in range((hi - lo) // 128):
                        t = (lo + k * 128) // 128
                        tr_ps = ppool.tile([128, d], fp32, tag="trps")
                        nc.tensor.transpose(
                            tr_ps[:], xwT_tmp[:, k * 128:(k + 1) * 128],
                            ident[:])
                        nc.vector.tensor_copy(xwbf[:, t, 0:d], tr_ps[:])

                # ---- AllGather the table ------------------------------------
                if split_ag:
                    ta = asplit // 128
                    nc.sync.dma_start(
                        xw_bounceA[:].rearrange("(t p) m -> p t m", p=128),
                        xwbf[:, 0:ta, :])
                    nc.sync.dma_start(
                        xw_bounceB[:].rearrange("(t p) m -> p t m", p=128),
                        xwbf[:, ta:nt, :])
                    if skip_ag:
                        nc.sync.dma_start(xw_fullA[0:asplit, :],
                                          xw_bounceA[:])
                        nc.sync.dma_start(xw_fullB[0:npad - asplit, :],
                                          xw_bounceB[:])
                    else:
                        nc.gpsimd.collective_compute(
                            "AllGather", mybir.AluOpType.bypass,
                            replica_groups=[list(range(n_cores))],
                            ins=[xw_bounceA.opt()], outs=[xw_fullA.opt()])
                        nc.gpsimd.collective_compute(
                            "AllGather", mybir.AluOpType.bypass,
                            replica_groups=[list(range(n_cores))],
                            ins=[xw_bounceB.opt()], outs=[xw_fullB.opt()])
                else:
                    nc.sync.dma_start(
                        xw_bounce1[:].rearrange("(t p) m -> p t m", p=128),
                        xwbf[:])
                    if skip_ag:
                        nc.sync.dma_start(xw_full1[0:npad, :], xw_bounce1[:])
                    else:
                        nc.gpsimd.collective_compute(
                            "AllGather", mybir.AluOpType.bypass,
                            replica_groups=[list(range(n_cores))],
                            ins=[xw_bounce1.opt()], outs=[xw_full1.opt()])

                # ---- aggregate: agg^T[:, blk] += msg^T @ S^T ----------------
                nc.vector.memset(p_sb[:], 0.0)
                agg_ps = {}
                for (tbl_ap, idx_sb, S_in, nch, sched) in (
                        (tblA_ap, iA_sb, Sa, nchA, schedA),
                        (tblB_ap, iB_sb, Sb, nchB, schedB)):
                    if skip_edges:
                        break
                    for c in range(nch):
                        msg = mpool.tile([128, subc, 128], bf16, tag="msg")
                        if skip_gather:
                            nc.vector.memset(msg[:], 0.0)
                        else:
                            nc.gpsimd.dma_gather(
                                msg[:], tbl_ap,
                                idx_sb[:, c * i16s:(c + 1) * i16s],
                                num_idxs=gchunk, num_idxs_reg=gchunk,
                                elem_size=128, queue_num=0,
                                single_packet=False)
                        if skip_smm:
                            continue
                        S_sb = stpool.tile([128, subc, blkn], bf16, tag="S")
                        nc.sync.dma_start(
                            S_sb[:],
                            S_in[:, c * subc * blkn:(c + 1) * subc * blkn]
                            .rearrange("e (s m) -> e s m", m=blkn))
                        for (j, blk, st, sp) in sched[c]:
                            if st:
                                agg_ps[blk] = apool.tile(
                                    [d, blkn], fp32, name="aggps", tag="aggps")
                            nc.tensor.matmul(agg_ps[blk][:],
                                             lhsT=msg[:, j, 0:d],
                                             rhs=S_sb[:, j, :],
                                             start=st, stop=sp)
                            if sp:
                                lo = blk * blkn
                                bw = min(blkn, npad - lo)
                                nc.vector.tensor_tensor(
                                    out=p_sb[:, lo:lo + bw],
                                    in0=p_sb[:, lo:lo + bw],
                                    in1=agg_ps.pop(blk)[:, :bw], op=Alu.add)

                # ---- epilogue (feature-major) ------------------------------
                nc.vector.tensor_scalar_add(p_sb[:], p_sb[:], bT_sb[:, i:i + 1])
                nc.vector.tensor_scalar_mul(t_sb[:], p_sb[:],
                                            prelu_sb[:d, i:i + 1])
                nc.vector.tensor_tensor(out=p_sb[:], in0=p_sb[:], in1=t_sb[:],
                                        op=Alu.max)
                if i > 0:
                    nc.vector.tensor_tensor(out=p_sb[:], in0=p_sb[:],
                                            in1=h_sb[:], op=Alu.add)
                nc.vector.tensor_tensor(out=t_sb[:], in0=p_sb[:], in1=p_sb[:],
                                        op=Alu.mult)
                nc.vector.reduce_sum(stat_sb[:, 0:1], p_sb[:, :nsh], axis=Ax.X)
                nc.vector.reduce_sum(stat_sb[:, 1:2], t_sb[:, :nsh], axis=Ax.X)
                nc.sync.dma_start(stats_in[:].rearrange("s d -> d s"),
                                  stat_sb[:])
                nc.gpsimd.collective_compute(
                    "AllReduce", mybir.AluOpType.add,
                    replica_groups=[list(range(n_cores))],
                    ins=[stats_in.opt()], outs=[stats_out.opt()])
                nc.sync.dma_start(stat2_sb[:],
                                  stats_out[:].rearrange("s d -> d s"))

                mean_c = rpool.tile([d, 1], fp32, tag="meanc")
                var_c = rpool.tile([d, 1], fp32, tag="varc")
                tmp_c = rpool.tile([d, 1], fp32, tag="tmpc")
                gs_c = rpool.tile([d, 1], fp32, tag="gsc")
                gb_c = rpool.tile([d, 1], fp32, tag="gbc")
                inv_n = 1.0 / float(n_nodes)
                nc.vector.tensor_scalar_mul(mean_c[:], stat2_sb[:, 0:1], inv_n)
                nc.vector.tensor_scalar_mul(var_c[:], stat2_sb[:, 1:2], inv_n)
                nc.vector.tensor_tensor(out=tmp_c[:], in0=mean_c[:],
                                        in1=mean_c[:], op=Alu.mult)
                nc.vector.tensor_tensor(out=var_c[:], in0=var_c[:],
                                        in1=tmp_c[:], op=Alu.subtract)
                nc.vector.tensor_scalar_add(var_c[:], var_c[:], BN_EPS)
                nc.scalar.activation(tmp_c[:], var_c[:],
                                     mybir.ActivationFunctionType.Sqrt)
                nc.vector.reciprocal(var_c[:], tmp_c[:])
                nc.vector.tensor_tensor(out=gs_c[:], in0=gaT_sb[:, i:i + 1],
                                        in1=var_c[:], op=Alu.mult)
                nc.vector.tensor_tensor(out=tmp_c[:], in0=mean_c[:],
                                        in1=gs_c[:], op=Alu.mult)
                nc.vector.tensor_tensor(out=gb_c[:], in0=beT_sb[:, i:i + 1],
                                        in1=tmp_c[:], op=Alu.subtract)
                nc.vector.tensor_scalar(out=h_sb[:], in0=p_sb[:],
                                        scalar1=gs_c[:], scalar2=gb_c[:],
                                        op0=Alu.mult, op1=Alu.add)

            for t in range(nt):
                tr_ps = ppool.tile([128, d], fp32, tag="trps")
                nc.tensor.transpose(tr_ps[:], h_sb[:, t * 128:(t + 1) * 128],
                                    ident[:])
                nc.vector.tensor_copy(out_sb[:, t, :], tr_ps[:])
            nc.sync.dma_start(out_ext[:].rearrange("(t p) d -> p t d", p=128),
                              out_sb[:])
    nc.compile()
    return nc


# ----------------------------------------------------------------------------
# Entry point
# ----------------------------------------------------------------------------

def kernel(x, edge_src, edge_dst, edge_weight, W, b, prelu_a,
           bn_gamma, bn_beta):
    from concourse.bass_utils import run_bass_kernel_spmd

    x = np.asarray(x)
    n = x.shape[0]
    nsh = n // N_CORES
    in_maps, cfg = _preprocess(x, edge_src, edge_dst, edge_weight, W, b,
                               prelu_a, bn_gamma, bn_beta,
                               N_CORES, nsh, GCHUNK, BLKN)
    nc = _build_nc(cfg)
    trace = bool(int(os.environ.get("GCN_TRACE", "0")))
    if trace:
        try:
            import antenv.axon_hooks  # noqa: F401
        except ImportError:
            trace = False
    res = run_bass_kernel_spmd(nc, in_maps, core_ids=list(range(N_CORES)),
                               trace=trace)
    LAST_RUN["results"] = res
    LAST_RUN["cfg"] = cfg
    LAST_RUN["nc"] = nc
    LAST_RUN["in_maps"] = in_maps
    out = np.concatenate(
        [res.results[r]["out"][:nsh] for r in range(N_CORES)], axis=0)
    return out.astype(np.float32)


def measure_exec_ns(nc, in_maps, n_reps=10):
    """Steady-state device-time estimate: pre-staged device inputs; marginal
    (slope) wall time of k back-to-back NEFF executions, amortizing the
    axon tunnel dispatch overhead."""
    import time
    import jax
    import concourse.mybir as mybir
    from jax.sharding import Mesh, PartitionSpec, NamedSharding
    from jax.experimental.shard_map import shard_map
    from concourse import bass2jax

    n_cores = len(in_maps)
    partition_name = (nc.partition_id_tensor.name
                      if nc.partition_id_tensor else None)
    in_names, out_names, out_avals = [], [], []
    for alloc in nc.m.functions[0].allocations:
        if not isinstance(alloc, mybir.MemoryLocationSet):
            continue
        name = alloc.memorylocations[0].name
        if alloc.kind == "ExternalInput":
            if name != partition_name:
                in_names.append(name)
        elif alloc.kind == "ExternalOutput":
            out_names.append(name)
            out_avals.append(jax.core.ShapedArray(
                tuple(alloc.tensor_shape), mybir.dt.np(alloc.dtype)))
    n_params = len(in_names)
    all_in = list(in_names) + list(out_names)
    if partition_name is not None:
        all_in.append(partition_name)

    def _body(*args):
        operands = list(args)
        if partition_name is not None:
            operands.append(bass2jax.partition_id_tensor())
        outs = bass2jax._bass_exec_p.bind(
            *operands, out_avals=tuple(out_avals), in_names=tuple(all_in),
            out_names=tuple(out_names), lowering_input_output_aliases=(),
            sim_require_finite=True, sim_require_nnan=True, nc=nc)
        return tuple(outs)

    devices = jax.devices()[:n_cores]
    mesh = Mesh(np.asarray(devices), ("core",))
    nin = n_params + len(out_names)
    fn = jax.jit(shard_map(_body, mesh=mesh,
                           in_specs=(PartitionSpec("core"),) * nin,
                           out_specs=(PartitionSpec("core"),) * len(out_names),
                           check_rep=False))
    sh = NamedSharding(mesh, PartitionSpec("core"))
    dev_in = [jax.device_put(
        np.concatenate([np.asarray(in_maps[c][k]) for c in range(n_cores)],
                       axis=0), sh) for k in in_names]
    dev_zero = [jax.device_put(
        np.zeros((n_cores * a.shape[0], *a.shape[1:]), a.dtype), sh)
        for a in out_avals]
    out = fn(*dev_in, *dev_zero)
    jax.block_until_ready(out)
    k = 16
    best1 = bestk = 1e9
    times = []
    for _ in range(n_reps):
        t0 = time.perf_counter()
        out = fn(*dev_in, *dev_zero)
        jax.block_until_ready(out)
        t = time.perf_counter() - t0
        times.append(t)
        best1 = min(best1, t)
    for _ in range(4):
        t0 = time.perf_counter()
        last = None
        for _ in range(k):
            last = fn(*dev_in, *dev_zero)
        jax.block_until_ready(last)
        bestk = min(bestk, time.perf_counter() - t0)
    marginal = (bestk - best1) / (k - 1)
    return int(marginal * 1e9), times


# revision 22
# speedup vs baseline: 1.0031x; 1.0031x over previous
"""GCN layers (3x GCNConv + PReLU + residual + BatchNorm) on 8 TRN2 NeuronCores.

Full-input contract: kernel(**inputs) takes unsharded numpy arrays and returns
the full [50000, 64] float32 output.

Sharding (graph/data parallel): nodes are partitioned into 8 contiguous
ranges; edges are bucketed to the core owning their dst node and grouped by
dst block. Per layer, per core:
  1. xw^T = W[i]^T @ h^T  (PE, feature-major), transposed+converted into a
     bf16 node-major gather table shard (padded rows of 128 cols, 64 valid)
  2. Two AllGathers build two table pieces in local HBM (the split keeps
     gather indices within int16 range, and lets table-A gathers overlap
     the second collective)
  3. stream edges: dma_gather 256B rows table[src] -> SBUF; aggregate with
     per-128-edge-subchunk segment matmuls agg^T[block] += msg^T @ S^T
     (S carries the edge weights, bf16, PSUM-accumulated per dst block)
  4. epilogue (feature-major [64, nodes]): +bias, PReLU, residual, BatchNorm
     with global batch stats via a tiny AllReduce; per-feature params are
     per-partition scalars (native tensor_scalar ops).

The per-block subchunk schedule is made uniform across cores (max-padded)
so all cores run the same SPMD program.
"""

import os
import numpy as np

N_NODES = 50000
D = 64
L = 3
BN_EPS = 1e-5
N_CORES = 8
GCHUNK = 6144           # edge slots per gather chunk (48 subchunks of 128)
BLKN = 512              # dst nodes per aggregation block (S columns)
IDX_LIMIT = 32768       # int16 gather index range

LAST_RUN = {}


# ----------------------------------------------------------------------------
# Host-side preprocessing
# ----------------------------------------------------------------------------

def _wrap16(flat, slots):
    """Edge-slot array -> [128, slots/16] int16 'wrapped' index layout."""
    a = flat.reshape(slots // 16, 16).T.astype(np.int16)
    return np.tile(a, (8, 1))


def _preprocess(x, edge_src, edge_dst, edge_weight, W, b, prelu_a,
                bn_gamma, bn_beta, n_cores, nsh, gchunk, blkn,
                split_ag=True):
    import ml_dtypes
    bf16 = ml_dtypes.bfloat16

    n = x.shape[0]
    d = x.shape[1]
    nt = (nsh + 127) // 128
    npad = nt * 128
    subc = gchunk // 128
    nblk = (npad + blkn - 1) // blkn
    # per-rank local-row split: table A = local rows [0, asplit), B = rest
    if npad * n_cores <= IDX_LIMIT:
        asplit = max(128, (npad // 2) // 128 * 128)
    else:
        asplit = (IDX_LIMIT // n_cores) // 128 * 128

    src = np.asarray(edge_src).astype(np.int64)
    dst = np.asarray(edge_dst).astype(np.int64)
    w = np.asarray(edge_weight).astype(np.float32)
    x = np.asarray(x).astype(np.float32)

    s_rank = src // nsh
    s_loc = src % nsh
    if split_ag:
        inA = s_loc < asplit
        idxA = s_rank * asplit + s_loc
        idxB = s_rank * (npad - asplit) + (s_loc - asplit)
    else:
        src_pad = s_rank * npad + s_loc
        gsplit = min(IDX_LIMIT, ((npad * n_cores) // 2) // 128 * 128)
        inA = src_pad < gsplit
        idxA = src_pad
        idxB = src_pad - gsplit
        asplit = gsplit  # reused as the global row split below
    shard = dst // nsh
    dst_local = dst % nsh

    streams = []
    for sel, tix in ((inA, idxA), (~inA, idxB)):
        per_core_edges = []
        cnts = []
        for r in range(n_cores):
            m = (shard == r) & sel
            per_core_edges.append((tix[m], dst_local[m], w[m]))
            cnts.append(np.bincount(dst_local[m] // blkn, minlength=nblk))
        nsub = np.zeros(nblk, np.int64)
        for c in cnts:
            nsub = np.maximum(nsub, (c + 127) // 128)
        sub_off = np.concatenate([[0], np.cumsum(nsub)])
        total_subs = int(sub_off[-1])
        nch = max(1, (total_subs + subc - 1) // subc)
        padded_subs = nch * subc
        slots = padded_subs * 128

        sched = []
        for blk in range(nblk):
            for j in range(int(nsub[blk])):
                gsub = int(sub_off[blk]) + j
                sched.append((gsub // subc, gsub % subc, blk,
                              j == 0, j == int(nsub[blk]) - 1))

        per_core = []
        for r in range(n_cores):
            ti, dl, wr = per_core_edges[r]
            blk = dl // blkn
            col = dl % blkn
            order = np.argsort(blk, kind="stable")
            ti, wr, blk, col = (a[order] for a in (ti, wr, blk, col))
            cnt = cnts[r]
            starts = np.concatenate([[0], np.cumsum(cnt)])
            pos = np.arange(len(ti)) - starts[blk]
            gsub = sub_off[blk] + pos // 128
            row = pos % 128
            slot = gsub * 128 + row
            rng_pad = np.random.default_rng(12345 + r)
            trows_s = int(ti.max()) + 1 if len(ti) else 1
            idx = rng_pad.integers(0, trows_s, slots)
            idx[slot] = ti
            S = np.zeros((padded_subs, 128, blkn), np.float32)
            S[gsub, row, col] = wr
            St = np.ascontiguousarray(S.transpose(1, 0, 2)
                                      .reshape(128, padded_subs * blkn))
            per_core.append((_wrap16(idx, slots), St.astype(bf16)))
        streams.append(dict(nch=nch, slots=slots, sched=sched,
                            per_core=per_core))

    bT = np.ascontiguousarray(np.asarray(b, np.float32).T)
    gammaT = np.ascontiguousarray(np.asarray(bn_gamma, np.float32).T)
    betaT = np.ascontiguousarray(np.asarray(bn_beta, np.float32).T)
    prelu_rep = np.tile(np.asarray(prelu_a, np.float32).reshape(1, L),
                        (128, 1))
    Wf = np.ascontiguousarray(np.asarray(W, np.float32))

    in_maps = []
    for r in range(n_cores):
        h0T = np.zeros((d, npad), np.float32)
        h0T[:, :nsh] = x[r * nsh:(r + 1) * nsh].T
        in_maps.append({
            "h0sT": h0T,
            "Wp": Wf,
            "bT": bT,
            "gammaT": gammaT,
            "betaT": betaT,
            "prelu_rep": prelu_rep,
            "srcA": streams[0]["per_core"][r][0],
            "Sa": streams[0]["per_core"][r][1],
            "srcB": streams[1]["per_core"][r][0],
            "Sb": streams[1]["per_core"][r][1],
        })

    cfg = dict(n_cores=n_cores, nsh=nsh, d=d, nt=nt, npad=npad,
               gchunk=gchunk, subc=subc, blkn=blkn, nblk=nblk,
               asplit=asplit, split_ag=split_ag, n_nodes=n,
               nchA=streams[0]["nch"], slotsA=streams[0]["slots"],
               schedA=streams[0]["sched"],
               nchB=streams[1]["nch"], slotsB=streams[1]["slots"],
               schedB=streams[1]["sched"])
    return in_maps, cfg


# ----------------------------------------------------------------------------
# Device program
# ----------------------------------------------------------------------------

def _build_nc(cfg):
    import concourse.bacc as bacc
    import concourse.tile as tile
    import concourse.mybir as mybir
    from concourse import library_config
    from concourse.masks import make_identity

    fp32 = mybir.dt.float32
    bf16 = mybir.dt.bfloat16
    i16 = mybir.dt.int16
    Alu = mybir.AluOpType
    Ax = mybir.AxisListType

    n_cores = cfg["n_cores"]
    nsh, d, nt, npad = cfg["nsh"], cfg["d"], cfg["nt"], cfg["npad"]
    gchunk, subc = cfg["gchunk"], cfg["subc"]
    blkn, nblk, asplit = cfg["blkn"], cfg["nblk"], cfg["asplit"]
    n_nodes = cfg["n_nodes"]
    slotsA, slotsB = cfg["slotsA"], cfg["slotsB"]
    nchA, nchB = cfg["nchA"], cfg["nchB"]
    split_ag = cfg.get("split_ag", True)
    if split_ag:
        rows_a = n_cores * asplit
        rows_b = n_cores * (npad - asplit)
    else:
        rows_a = asplit
        rows_b = n_cores * npad - asplit
    i16s = gchunk // 16
    nxc = (npad + 511) // 512

    def by_chunk(sched, nch):
        per = [[] for _ in range(nch)]
        for (c, j, blk, st, sp) in sched:
            per[c].append((j, blk, st, sp))
        return per

    schedA = by_chunk(cfg["schedA"], nchA)
    schedB = by_chunk(cfg["schedB"], nchB)
    skip_gather = cfg.get("skip_gather", False)
    skip_smm = cfg.get("skip_smm", False)
    skip_edges = cfg.get("skip_edges", False)
    skip_ag = cfg.get("skip_ag", False)

    nc = bacc.Bacc(None, target_bir_lowering=False, debug=False)

    h0sT = nc.declare_dram_parameter("h0sT", [d, npad], fp32, isOutput=False)
    Wp = nc.declare_dram_parameter("Wp", [L, d, d], fp32, isOutput=False)
    bT_in = nc.declare_dram_parameter("bT", [d, L], fp32, isOutput=False)
    gammaT_in = nc.declare_dram_parameter("gammaT", [d, L], fp32, isOutput=False)
    betaT_in = nc.declare_dram_parameter("betaT", [d, L], fp32, isOutput=False)
    prelu_in = nc.declare_dram_parameter("prelu_rep", [128, L], fp32, isOutput=False)
    srcA = nc.declare_dram_parameter("srcA", [128, slotsA // 16], i16, isOutput=False)
    Sa = nc.declare_dram_parameter("Sa", [128, slotsA * blkn // 128], bf16, isOutput=False)
    srcB = nc.declare_dram_parameter("srcB", [128, slotsB // 16], i16, isOutput=False)
    Sb = nc.declare_dram_parameter("Sb", [128, slotsB * blkn // 128], bf16, isOutput=False)
    out_ext = nc.declare_dram_parameter("out", [npad, d], fp32, isOutput=True)

    with tile.TileContext(nc) as tc:
        with (
            tc.tile_pool(name="const", bufs=1) as cpool,
            tc.tile_pool(name="state", bufs=1) as spool,
            tc.tile_pool(name="meta", bufs=1) as epool,
            tc.tile_pool(name="work", bufs=2) as wpool,
            tc.tile_pool(name="rows", bufs=2) as rpool,
            tc.tile_pool(name="msg", bufs=2) as mpool,
            tc.tile_pool(name="smat", bufs=2) as stpool,
            tc.tile_pool(name="ps", bufs=2, space="PSUM") as ppool,
            tc.tile_pool(name="psagg", bufs=2, space="PSUM") as apool,
            tc.tile_pool(name="dram", bufs=1, space="DRAM") as dpool,
        ):
            h_sb = spool.tile([d, npad], fp32, tag="h")
            p_sb = spool.tile([d, npad], fp32, tag="p")
            t_sb = spool.tile([d, npad], fp32, tag="t")
            xwbf = spool.tile([128, nt, 128], bf16, tag="xwbf")
            out_sb = spool.tile([128, nt, d], fp32, tag="osb")
            ident = cpool.tile([d, d], fp32, tag="ident")
            W_sb = cpool.tile([d, L * d], fp32, tag="Wsb")
            bT_sb = cpool.tile([d, L], fp32, tag="bT")
            gaT_sb = cpool.tile([d, L], fp32, tag="gaT")
            beT_sb = cpool.tile([d, L], fp32, tag="beT")
            prelu_sb = cpool.tile([128, L], fp32, tag="prelu")
            stat_sb = spool.tile([d, 2], fp32, tag="stat")
            stat2_sb = spool.tile([d, 2], fp32, tag="stat2")
            iA_sb = epool.tile([128, slotsA // 16], i16, tag="iA")
            iB_sb = epool.tile([128, slotsB // 16], i16, tag="iB")

            if split_ag:
                xw_bounceA = dpool.tile([asplit, 128], bf16, tag="xwbA")
                xw_bounceB = dpool.tile([npad - asplit, 128], bf16, tag="xwbB")
                xw_fullA = dpool.tile([rows_a, 128], bf16, tag="xwfA")
                xw_fullB = dpool.tile([rows_b, 128], bf16, tag="xwfB")
                tblA_ap, tblB_ap = xw_fullA[:], xw_fullB[:]
            else:
                xw_bounce1 = dpool.tile([npad, 128], bf16, tag="xwb1")
                xw_full1 = dpool.tile([n_cores * npad, 128], bf16, tag="xwf1")
                tblA_ap = xw_full1[0:rows_a, :]
                tblB_ap = xw_full1[rows_a:n_cores * npad, :]
            stats_in = dpool.tile([2, d], fp32, tag="sin")
            stats_out = dpool.tile([2, d], fp32, tag="sout")

            nc.sync.dma_start(h_sb[:], h0sT[:])
            for i in range(L):
                nc.sync.dma_start(W_sb[:, i * d:(i + 1) * d], Wp[i, :, :])
            nc.sync.dma_start(bT_sb[:], bT_in[:])
            nc.sync.dma_start(gaT_sb[:], gammaT_in[:])
            nc.sync.dma_start(beT_sb[:], betaT_in[:])
            nc.sync.dma_start(prelu_sb[:], prelu_in[:])
            nc.sync.dma_start(iA_sb[:], srcA[:])
            nc.sync.dma_start(iB_sb[:], srcB[:])
            make_identity(nc, ident[:])
            nc.vector.memset(xwbf[:], 0.0)
            nc.gpsimd.load_library(library_config.mlp)

            for i in range(L):
                # ---- xw^T = W[i]^T @ h^T, transpose+convert -----------------
                for c in range(nxc):
                    lo = c * 512
                    hi = min(npad, lo + 512)
                    xw_ps = ppool.tile([d, 512], fp32, tag="xwps")
                    nc.tensor.matmul(xw_ps[:, :hi - lo],
                                     lhsT=W_sb[:, i * d:(i + 1) * d],
                                     rhs=h_sb[:, lo:hi], start=True, stop=True)
                    xwT_tmp = wpool.tile([d, 512], fp32, tag="xwt")
                    nc.vector.tensor_copy(xwT_tmp[:, :hi - lo],
                                          xw_ps[:, :hi - lo])
                    for k in range((hi - lo) // 128):
                        t = (lo + k * 128) // 128
                        tr_ps = ppool.tile([128, d], fp32, tag="trps")
                        nc.tensor.transpose(
                            tr_ps[:], xwT_tmp[:, k * 128:(k + 1) * 128],
                            ident[:])
                        nc.vector.tensor_copy(xwbf[:, t, 0:d], tr_ps[:])

                # ---- AllGather the table ------------------------------------
                if split_ag:
                    ta = asplit // 128
                    nc.sync.dma_start(
                        xw_bounceA[:].rearrange("(t p) m -> p t m", p=128),
                        xwbf[:, 0:ta, :])
                    nc.sync.dma_start(
                        xw_bounceB[:].rearrange("(t p) m -> p t m", p=128),
                        xwbf[:, ta:nt, :])
                    if skip_ag:
                        nc.sync.dma_start(xw_fullA[0:asplit, :],
                                          xw_bounceA[:])
                        nc.sync.dma_start(xw_fullB[0:npad - asplit, :],
                                          xw_bounceB[:])
                    else:
                        nc.gpsimd.collective_compute(
                            "AllGather", mybir.AluOpType.bypass,
                            replica_groups=[list(range(n_cores))],
                            ins=[xw_bounceA.opt()], outs=[xw_fullA.opt()])
                        nc.gpsimd.collective_compute(
                            "AllGather", mybir.AluOpType.bypass,
                            replica_groups=[list(range(n_cores))],
                            ins=[xw_bounceB.opt()], outs=[xw_fullB.opt()])
                else:
                    nc.sync.dma_start(
                        xw_bounce1[:].rearrange("(t p) m -> p t m", p=128),
                        xwbf[:])
                    if skip_ag:
                        nc.sync.dma_start(xw_full1[0:npad, :], xw_bounce1[:])
                    else:
                        nc.gpsimd.collective_compute(
                            "AllGather", mybir.AluOpType.bypass,
                            replica_groups=[list(range(n_cores))],
                            ins=[xw_bounce1.opt()], outs=[xw_full1.opt()])

                # ---- aggregate: agg^T[:, blk] += msg^T @ S^T ----------------
                nc.vector.memset(p_sb[:], 0.0)
                agg_ps = {}
                for (tbl_ap, idx_sb, S_in, nch, sched) in (
                        (tblA_ap, iA_sb, Sa, nchA, schedA),
                        (tblB_ap, iB_sb, Sb, nchB, schedB)):
                    if skip_edges:
                        break
                    for c in range(nch):
                        msg = mpool.tile([128, subc, 128], bf16, tag="msg")
                        if skip_gather:
                            nc.vector.memset(msg[:], 0.0)
                        else:
                            nc.gpsimd.dma_gather(
                                msg[:], tbl_ap,
                                idx_sb[:, c * i16s:(c + 1) * i16s],
                                num_idxs=gchunk, num_idxs_reg=gchunk,
                                elem_size=128, queue_num=0,
                                single_packet=False)
                        if skip_smm:
                            continue
                        hs = subc // 2
                        S_half = []
                        for h in range(2):
                            base = (c * subc + h * hs) * blkn
                            S_sb = stpool.tile([128, hs, blkn], bf16,
                                               name="S_sb", tag="S")
                            nc.sync.dma_start(
                                S_sb[:],
                                S_in[:, base:base + hs * blkn]
                                .rearrange("e (s m) -> e s m", m=blkn))
                            S_half.append(S_sb)
                        for (j, blk, st, sp) in sched[c]:
                            if st:
                                agg_ps[blk] = apool.tile(
                                    [d, blkn], fp32, name="aggps", tag="aggps")
                            nc.tensor.matmul(agg_ps[blk][:],
                                             lhsT=msg[:, j, 0:d],
                                             rhs=S_half[j // hs][:, j % hs, :],
                                             start=st, stop=sp)
                            if sp:
                                lo = blk * blkn
                                bw = min(blkn, npad - lo)
                                nc.vector.tensor_tensor(
                                    out=p_sb[:, lo:lo + bw],
                                    in0=p_sb[:, lo:lo + bw],
                                    in1=agg_ps.pop(blk)[:, :bw], op=Alu.add)

                # ---- epilogue (feature-major) ------------------------------
                nc.vector.tensor_scalar_add(p_sb[:], p_sb[:], bT_sb[:, i:i + 1])
                nc.vector.tensor_scalar_mul(t_sb[:], p_sb[:],
                                            prelu_sb[:d, i:i + 1])
                nc.vector.tensor_tensor(out=p_sb[:], in0=p_sb[:], in1=t_sb[:],
                                        op=Alu.max)
                if i > 0:
                    nc.vector.tensor_tensor(out=p_sb[:], in0=p_sb[:],
                                            in1=h_sb[:], op=Alu.add)
                nc.vector.tensor_tensor(out=t_sb[:], in0=p_sb[:], in1=p_sb[:],
                                        op=Alu.mult)
                nc.vector.reduce_sum(stat_sb[:, 0:1], p_sb[:, :nsh], axis=Ax.X)
                nc.vector.reduce_sum(stat_sb[:, 1:2], t_sb[:, :nsh], axis=Ax.X)
                nc.sync.dma_start(stats_in[:].rearrange("s d -> d s"),
                                  stat_sb[:])
                nc.gpsimd.collective_compute(
                    "AllReduce", mybir.AluOpType.add,
                    replica_groups=[list(range(n_cores))],
                    ins=[stats_in.opt()], outs=[stats_out.opt()])
                nc.sync.dma_start(stat2_sb[:],
                                  stats_out[:].rearrange("s d -> d s"))

                mean_c = rpool.tile([d, 1], fp32, tag="meanc")
                var_c = rpool.tile([d, 1], fp32, tag="varc")
                tmp_c = rpool.tile([d, 1], fp32, tag="tmpc")
                gs_c = rpool.tile([d, 1], fp32, tag="gsc")
                gb_c = rpool.tile([d, 1], fp32, tag="gbc")
                inv_n = 1.0 / float(n_nodes)
                nc.vector.tensor_scalar_mul(mean_c[:], stat2_sb[:, 0:1], inv_n)
                nc.vector.tensor_scalar_mul(var_c[:], stat2_sb[:, 1:2], inv_n)
                nc.vector.tensor_tensor(out=tmp_c[:], in0=mean_c[:],
                                        in1=mean_c[:], op=Alu.mult)
                nc.vector.tensor_tensor(out=var_c[:], in0=var_c[:],
                                        in1=tmp_c[:], op=Alu.subtract)
                nc.vector.tensor_scalar_add(var_c[:], var_c[:], BN_EPS)
                nc.scalar.activation(tmp_c[:], var_c[:],
                                     mybir.ActivationFunctionType.Sqrt)
                nc.vector.reciprocal(var_c[:], tmp_c[:])
                nc.vector.tensor_tensor(out=gs_c[:], in0=gaT_sb[:, i:i + 1],
                                        in1=var_c[:], op=Alu.mult)
                nc.vector.tensor_tensor(out=tmp_c[:], in0=mean_c[:],
                                        in1=gs_c[:], op=Alu.mult)
                nc.vector.tensor_tensor(out=gb_c[:], in0=beT_sb[:, i:i + 1],
                                        in1=tmp_c[:], op=Alu.subtract)
                nc.vector.tensor_scalar(out=h_sb[:], in0=p_sb[:],
                                        scalar1=gs_c[:], scalar2=gb_c[:],
                                        op0=Alu.mult, op1=Alu.add)

            for t in range(nt):
                tr_ps = ppool.tile([128, d], fp32, tag="trps")
                nc.tensor.transpose(tr_ps[:], h_sb[:, t * 128:(t + 1) * 128],
                                    ident[:])
                nc.vector.tensor_copy(out_sb[:, t, :], tr_ps[:])
            nc.sync.dma_start(out_ext[:].rearrange("(t p) d -> p t d", p=128),
                              out_sb[:])
    nc.compile()
    return nc


# ----------------------------------------------------------------------------
# Entry point
# ----------------------------------------------------------------------------

def kernel(x, edge_src, edge_dst, edge_weight, W, b, prelu_a,
           bn_gamma, bn_beta):
    from concourse.bass_utils import run_bass_kernel_spmd

    x = np.asarray(x)
    n = x.shape[0]
    nsh = n // N_CORES
    in_maps, cfg = _preprocess(x, edge_src, edge_dst, edge_weight, W, b,
                               prelu_a, bn_gamma, bn_beta,
                               N_CORES, nsh, GCHUNK, BLKN)
    nc = _build_nc(cfg)
    trace = bool(int(os.environ.get("GCN_TRACE", "0")))
    if trace:
        try:
            import antenv.axon_hooks  # noqa: F401
        except ImportError:
            trace = False
    res = run_bass_kernel_spmd(nc, in_maps, core_ids=list(range(N_CORES)),
                               trace=trace)
    LAST_RUN["results"] = res
    LAST_RUN["cfg"] = cfg
    LAST_RUN["nc"] = nc
    LAST_RUN["in_maps"] = in_maps
    out = np.concatenate(
        [res.results[r]["out"][:nsh] for r in range(N_CORES)], axis=0)
    return out.astype(np.float32)


def measure_exec_ns(nc, in_maps, n_reps=10):
    """Steady-state device-time estimate: pre-staged device inputs; marginal
    (slope) wall time of k back-to-back NEFF executions, amortizing the
    axon tunnel dispatch overhead."""
    import time
    import jax
    import concourse.mybir as mybir
    from jax.sharding import Mesh, PartitionSpec, NamedSharding
    from jax.experimental.shard_map import shard_map
    from concourse import bass2jax

    n_cores = len(in_maps)
    partition_name = (nc.partition_id_tensor.name
                      if nc.partition_id_tensor else None)
    in_names, out_names, out_avals = [], [], []
    for alloc in nc.m.functions[0].allocations:
        if not isinstance(alloc, mybir.MemoryLocationSet):
            continue
        name = alloc.memorylocations[0].name
        if alloc.kind == "ExternalInput":
            if name != partition_name:
                in_names.append(name)
        elif alloc.kind == "ExternalOutput":
            out_names.append(name)
            out_avals.append(jax.core.ShapedArray(
                tuple(alloc.tensor_shape), mybir.dt.np(alloc.dtype)))
    n_params = len(in_names)
    all_in = list(in_names) + list(out_names)
    if partition_name is not None:
        all_in.append(partition_name)

    def _body(*args):
        operands = list(args)
        if partition_name is not None:
            operands.append(bass2jax.partition_id_tensor())
        outs = bass2jax._bass_exec_p.bind(
            *operands, out_avals=tuple(out_avals), in_names=tuple(all_in),
            out_names=tuple(out_names), lowering_input_output_aliases=(),
            sim_require_finite=True, sim_require_nnan=True, nc=nc)
        return tuple(outs)

    devices = jax.devices()[:n_cores]
    mesh = Mesh(np.asarray(devices), ("core",))
    nin = n_params + len(out_names)
    fn = jax.jit(shard_map(_body, mesh=mesh,
                           in_specs=(PartitionSpec("core"),) * nin,
                           out_specs=(PartitionSpec("core"),) * len(out_names),
                           check_rep=False))
    sh = NamedSharding(mesh, PartitionSpec("core"))
    dev_in = [jax.device_put(
        np.concatenate([np.asarray(in_maps[c][k]) for c in range(n_cores)],
                       axis=0), sh) for k in in_names]
    dev_zero = [jax.device_put(
        np.zeros((n_cores * a.shape[0], *a.shape[1:]), a.dtype), sh)
        for a in out_avals]
    out = fn(*dev_in, *dev_zero)
    jax.block_until_ready(out)
    k = 16
    best1 = bestk = 1e9
    times = []
    for _ in range(n_reps):
        t0 = time.perf_counter()
        out = fn(*dev_in, *dev_zero)
        jax.block_until_ready(out)
        t = time.perf_counter() - t0
        times.append(t)
        best1 = min(best1, t)
    for _ in range(4):
        t0 = time.perf_counter()
        last = None
        for _ in range(k):
            last = fn(*dev_in, *dev_zero)
        jax.block_until_ready(last)
        bestk = min(bestk, time.perf_counter() - t0)
    marginal = (bestk - best1) / (k - 1)
    return int(marginal * 1e9), times


# revision 23
# speedup vs baseline: 1.0524x; 1.0492x over previous
"""GCN layers (3x GCNConv + PReLU + residual + BatchNorm) on 8 TRN2 NeuronCores.

Full-input contract: kernel(**inputs) takes unsharded numpy arrays and returns
the full [50000, 64] float32 output.

Sharding (graph/data parallel): nodes are partitioned into 8 contiguous
ranges; edges are bucketed to the core owning their dst node and grouped by
dst block. Per layer, per core:
  1. xw^T = W[i]^T @ h^T  (PE, feature-major), transposed+converted into a
     bf16 node-major gather table shard (padded rows of 128 cols, 64 valid)
  2. Two AllGathers build two table pieces in local HBM (the split keeps
     gather indices within int16 range, and lets table-A gathers overlap
     the second collective)
  3. stream edges: dma_gather 256B rows table[src] -> SBUF; aggregate with
     per-128-edge-subchunk segment matmuls agg^T[block] += msg^T @ S^T
     (S carries the edge weights, bf16, PSUM-accumulated per dst block)
  4. epilogue (feature-major [64, nodes]): +bias, PReLU, residual, BatchNorm
     with global batch stats via a tiny AllReduce; per-feature params are
     per-partition scalars (native tensor_scalar ops).

The per-block subchunk schedule is made uniform across cores (max-padded)
so all cores run the same SPMD program.
"""

import os
import numpy as np

N_NODES = 50000
D = 64
L = 3
BN_EPS = 1e-5
N_CORES = 8
GCHUNK = 6144           # edge slots per gather chunk (48 subchunks of 128)
BLKN = 512              # dst nodes per aggregation block (S columns)
IDX_LIMIT = 32768       # int16 gather index range

LAST_RUN = {}


# ----------------------------------------------------------------------------
# Host-side preprocessing
# ----------------------------------------------------------------------------

def _wrap16(flat, slots):
    """Edge-slot array -> [128, slots/16] int16 'wrapped' index layout."""
    a = flat.reshape(slots // 16, 16).T.astype(np.int16)
    return np.tile(a, (8, 1))


def _preprocess(x, edge_src, edge_dst, edge_weight, W, b, prelu_a,
                bn_gamma, bn_beta, n_cores, nsh, gchunk, blkn,
                split_ag=True):
    import ml_dtypes
    bf16 = ml_dtypes.bfloat16

    n = x.shape[0]
    d = x.shape[1]
    nt = (nsh + 127) // 128
    npad = nt * 128
    subc = gchunk // 128
    nblk = (npad + blkn - 1) // blkn
    # per-rank local-row split: table A = local rows [0, asplit), B = rest
    if npad * n_cores <= IDX_LIMIT:
        asplit = max(128, (npad // 2) // 128 * 128)
    else:
        asplit = (IDX_LIMIT // n_cores) // 128 * 128

    src = np.asarray(edge_src).astype(np.int64)
    dst = np.asarray(edge_dst).astype(np.int64)
    w = np.asarray(edge_weight).astype(np.float32)
    x = np.asarray(x).astype(np.float32)

    s_rank = src // nsh
    s_loc = src % nsh
    if split_ag:
        inA = s_loc < asplit
        idxA = s_rank * asplit + s_loc
        idxB = s_rank * (npad - asplit) + (s_loc - asplit)
    else:
        src_pad = s_rank * npad + s_loc
        gsplit = min(IDX_LIMIT, ((npad * n_cores) // 2) // 128 * 128)
        inA = src_pad < gsplit
        idxA = src_pad
        idxB = src_pad - gsplit
        asplit = gsplit  # reused as the global row split below
    shard = dst // nsh
    dst_local = dst % nsh

    streams = []
    for sel, tix in ((inA, idxA), (~inA, idxB)):
        per_core_edges = []
        cnts = []
        for r in range(n_cores):
            m = (shard == r) & sel
            per_core_edges.append((tix[m], dst_local[m], w[m]))
            cnts.append(np.bincount(dst_local[m] // blkn, minlength=nblk))
        nsub = np.zeros(nblk, np.int64)
        for c in cnts:
            nsub = np.maximum(nsub, (c + 127) // 128)
        sub_off = np.concatenate([[0], np.cumsum(nsub)])
        total_subs = int(sub_off[-1])
        nch = max(1, (total_subs + subc - 1) // subc)
        padded_subs = nch * subc
        slots = padded_subs * 128

        sched = []
        for blk in range(nblk):
            for j in range(int(nsub[blk])):
                gsub = int(sub_off[blk]) + j
                sched.append((gsub // subc, gsub % subc, blk,
                              j == 0, j == int(nsub[blk]) - 1))

        per_core = []
        for r in range(n_cores):
            ti, dl, wr = per_core_edges[r]
            blk = dl // blkn
            col = dl % blkn
            order = np.argsort(blk, kind="stable")
            ti, wr, blk, col = (a[order] for a in (ti, wr, blk, col))
            cnt = cnts[r]
            starts = np.concatenate([[0], np.cumsum(cnt)])
            pos = np.arange(len(ti)) - starts[blk]
            gsub = sub_off[blk] + pos // 128
            row = pos % 128
            slot = gsub * 128 + row
            rng_pad = np.random.default_rng(12345 + r)
            trows_s = int(ti.max()) + 1 if len(ti) else 1
            idx = rng_pad.integers(0, trows_s, slots)
            idx[total_subs * 128:] = -1   # uniform tail dummies: no descriptors
            idx[slot] = ti
            S = np.zeros((padded_subs, 128, blkn), np.float32)
            S[gsub, row, col] = wr
            St = np.ascontiguousarray(S.transpose(1, 0, 2)
                                      .reshape(128, padded_subs * blkn))
            per_core.append((_wrap16(idx, slots), St.astype(bf16)))
        streams.append(dict(nch=nch, slots=slots, sched=sched,
                            total_subs=total_subs, per_core=per_core))

    bT = np.ascontiguousarray(np.asarray(b, np.float32).T)
    gammaT = np.ascontiguousarray(np.asarray(bn_gamma, np.float32).T)
    betaT = np.ascontiguousarray(np.asarray(bn_beta, np.float32).T)
    prelu_rep = np.tile(np.asarray(prelu_a, np.float32).reshape(1, L),
                        (128, 1))
    Wf = np.ascontiguousarray(np.asarray(W, np.float32))

    in_maps = []
    for r in range(n_cores):
        h0T = np.zeros((d, npad), np.float32)
        h0T[:, :nsh] = x[r * nsh:(r + 1) * nsh].T
        in_maps.append({
            "h0sT": h0T,
            "Wp": Wf,
            "bT": bT,
            "gammaT": gammaT,
            "betaT": betaT,
            "prelu_rep": prelu_rep,
            "srcA": streams[0]["per_core"][r][0],
            "Sa": streams[0]["per_core"][r][1],
            "srcB": streams[1]["per_core"][r][0],
            "Sb": streams[1]["per_core"][r][1],
        })

    cfg = dict(n_cores=n_cores, nsh=nsh, d=d, nt=nt, npad=npad,
               gchunk=gchunk, subc=subc, blkn=blkn, nblk=nblk,
               asplit=asplit, split_ag=split_ag, n_nodes=n,
               nchA=streams[0]["nch"], slotsA=streams[0]["slots"],
               schedA=streams[0]["sched"], subsA=streams[0]["total_subs"],
               nchB=streams[1]["nch"], slotsB=streams[1]["slots"],
               schedB=streams[1]["sched"], subsB=streams[1]["total_subs"])
    return in_maps, cfg


# ----------------------------------------------------------------------------
# Device program
# ----------------------------------------------------------------------------

def _build_nc(cfg):
    import concourse.bacc as bacc
    import concourse.tile as tile
    import concourse.mybir as mybir
    from concourse import library_config
    from concourse.masks import make_identity

    fp32 = mybir.dt.float32
    bf16 = mybir.dt.bfloat16
    i16 = mybir.dt.int16
    Alu = mybir.AluOpType
    Ax = mybir.AxisListType

    n_cores = cfg["n_cores"]
    nsh, d, nt, npad = cfg["nsh"], cfg["d"], cfg["nt"], cfg["npad"]
    gchunk, subc = cfg["gchunk"], cfg["subc"]
    blkn, nblk, asplit = cfg["blkn"], cfg["nblk"], cfg["asplit"]
    n_nodes = cfg["n_nodes"]
    slotsA, slotsB = cfg["slotsA"], cfg["slotsB"]
    nchA, nchB = cfg["nchA"], cfg["nchB"]
    split_ag = cfg.get("split_ag", True)
    if split_ag:
        rows_a = n_cores * asplit
        rows_b = n_cores * (npad - asplit)
    else:
        rows_a = asplit
        rows_b = n_cores * npad - asplit
    i16s = gchunk // 16
    nxc = (npad + 511) // 512

    def by_chunk(sched, nch):
        per = [[] for _ in range(nch)]
        for (c, j, blk, st, sp) in sched:
            per[c].append((j, blk, st, sp))
        return per

    schedA = by_chunk(cfg["schedA"], nchA)
    schedB = by_chunk(cfg["schedB"], nchB)
    skip_gather = cfg.get("skip_gather", False)
    skip_smm = cfg.get("skip_smm", False)
    skip_edges = cfg.get("skip_edges", False)
    skip_ag = cfg.get("skip_ag", False)

    nc = bacc.Bacc(None, target_bir_lowering=False, debug=False)

    h0sT = nc.declare_dram_parameter("h0sT", [d, npad], fp32, isOutput=False)
    Wp = nc.declare_dram_parameter("Wp", [L, d, d], fp32, isOutput=False)
    bT_in = nc.declare_dram_parameter("bT", [d, L], fp32, isOutput=False)
    gammaT_in = nc.declare_dram_parameter("gammaT", [d, L], fp32, isOutput=False)
    betaT_in = nc.declare_dram_parameter("betaT", [d, L], fp32, isOutput=False)
    prelu_in = nc.declare_dram_parameter("prelu_rep", [128, L], fp32, isOutput=False)
    srcA = nc.declare_dram_parameter("srcA", [128, slotsA // 16], i16, isOutput=False)
    Sa = nc.declare_dram_parameter("Sa", [128, slotsA * blkn // 128], bf16, isOutput=False)
    srcB = nc.declare_dram_parameter("srcB", [128, slotsB // 16], i16, isOutput=False)
    Sb = nc.declare_dram_parameter("Sb", [128, slotsB * blkn // 128], bf16, isOutput=False)
    out_ext = nc.declare_dram_parameter("out", [npad, d], fp32, isOutput=True)

    with tile.TileContext(nc) as tc:
        with (
            tc.tile_pool(name="const", bufs=1) as cpool,
            tc.tile_pool(name="state", bufs=1) as spool,
            tc.tile_pool(name="meta", bufs=1) as epool,
            tc.tile_pool(name="work", bufs=2) as wpool,
            tc.tile_pool(name="rows", bufs=2) as rpool,
            tc.tile_pool(name="msg", bufs=2) as mpool,
            tc.tile_pool(name="smat", bufs=2) as stpool,
            tc.tile_pool(name="ps", bufs=2, space="PSUM") as ppool,
            tc.tile_pool(name="psagg", bufs=2, space="PSUM") as apool,
            tc.tile_pool(name="dram", bufs=1, space="DRAM") as dpool,
        ):
            h_sb = spool.tile([d, npad], fp32, tag="h")
            p_sb = spool.tile([d, npad], fp32, tag="p")
            t_sb = spool.tile([d, npad], fp32, tag="t")
            xwbf = spool.tile([128, nt, 128], bf16, tag="xwbf")
            out_sb = spool.tile([128, nt, d], fp32, tag="osb")
            ident = cpool.tile([d, d], fp32, tag="ident")
            W_sb = cpool.tile([d, L * d], fp32, tag="Wsb")
            bT_sb = cpool.tile([d, L], fp32, tag="bT")
            gaT_sb = cpool.tile([d, L], fp32, tag="gaT")
            beT_sb = cpool.tile([d, L], fp32, tag="beT")
            prelu_sb = cpool.tile([128, L], fp32, tag="prelu")
            stat_sb = spool.tile([d, 2], fp32, tag="stat")
            stat2_sb = spool.tile([d, 2], fp32, tag="stat2")
            iA_sb = epool.tile([128, slotsA // 16], i16, tag="iA")
            iB_sb = epool.tile([128, slotsB // 16], i16, tag="iB")

            if split_ag:
                xw_bounceA = dpool.tile([asplit, 128], bf16, tag="xwbA")
                xw_bounceB = dpool.tile([npad - asplit, 128], bf16, tag="xwbB")
                xw_fullA = dpool.tile([rows_a, 128], bf16, tag="xwfA")
                xw_fullB = dpool.tile([rows_b, 128], bf16, tag="xwfB")
                tblA_ap, tblB_ap = xw_fullA[:], xw_fullB[:]
            else:
                xw_bounce1 = dpool.tile([npad, 128], bf16, tag="xwb1")
                xw_full1 = dpool.tile([n_cores * npad, 128], bf16, tag="xwf1")
                tblA_ap = xw_full1[0:rows_a, :]
                tblB_ap = xw_full1[rows_a:n_cores * npad, :]
            stats_in = dpool.tile([2, d], fp32, tag="sin")
            stats_out = dpool.tile([2, d], fp32, tag="sout")

            nc.sync.dma_start(h_sb[:], h0sT[:])
            for i in range(L):
                nc.sync.dma_start(W_sb[:, i * d:(i + 1) * d], Wp[i, :, :])
            nc.sync.dma_start(bT_sb[:], bT_in[:])
            nc.sync.dma_start(gaT_sb[:], gammaT_in[:])
            nc.sync.dma_start(beT_sb[:], betaT_in[:])
            nc.sync.dma_start(prelu_sb[:], prelu_in[:])
            nc.sync.dma_start(iA_sb[:], srcA[:])
            nc.sync.dma_start(iB_sb[:], srcB[:])
            make_identity(nc, ident[:])
            nc.vector.memset(xwbf[:], 0.0)
            nc.gpsimd.load_library(library_config.mlp)

            for i in range(L):
                # ---- xw^T = W[i]^T @ h^T, transpose+convert -----------------
                for c in range(nxc):
                    lo = c * 512
                    hi = min(npad, lo + 512)
                    xw_ps = ppool.tile([d, 512], fp32, tag="xwps")
                    nc.tensor.matmul(xw_ps[:, :hi - lo],
                                     lhsT=W_sb[:, i * d:(i + 1) * d],
                                     rhs=h_sb[:, lo:hi], start=True, stop=True)
                    xwT_tmp = wpool.tile([d, 512], fp32, tag="xwt")
                    nc.vector.tensor_copy(xwT_tmp[:, :hi - lo],
                                          xw_ps[:, :hi - lo])
                    for k in range((hi - lo) // 128):
                        t = (lo + k * 128) // 128
                        tr_ps = ppool.tile([128, d], fp32, tag="trps")
                        nc.tensor.transpose(
                            tr_ps[:], xwT_tmp[:, k * 128:(k + 1) * 128],
                            ident[:])
                        nc.vector.tensor_copy(xwbf[:, t, 0:d], tr_ps[:])

                # ---- AllGather the table ------------------------------------
                if split_ag:
                    ta = asplit // 128
                    nc.sync.dma_start(
                        xw_bounceA[:].rearrange("(t p) m -> p t m", p=128),
                        xwbf[:, 0:ta, :])
                    nc.sync.dma_start(
                        xw_bounceB[:].rearrange("(t p) m -> p t m", p=128),
                        xwbf[:, ta:nt, :])
                    if skip_ag:
                        nc.sync.dma_start(xw_fullA[0:asplit, :],
                                          xw_bounceA[:])
                        nc.sync.dma_start(xw_fullB[0:npad - asplit, :],
                                          xw_bounceB[:])
                    else:
                        nc.gpsimd.collective_compute(
                            "AllGather", mybir.AluOpType.bypass,
                            replica_groups=[list(range(n_cores))],
                            ins=[xw_bounceA.opt()], outs=[xw_fullA.opt()])
                        nc.gpsimd.collective_compute(
                            "AllGather", mybir.AluOpType.bypass,
                            replica_groups=[list(range(n_cores))],
                            ins=[xw_bounceB.opt()], outs=[xw_fullB.opt()])
                else:
                    nc.sync.dma_start(
                        xw_bounce1[:].rearrange("(t p) m -> p t m", p=128),
                        xwbf[:])
                    if skip_ag:
                        nc.sync.dma_start(xw_full1[0:npad, :], xw_bounce1[:])
                    else:
                        nc.gpsimd.collective_compute(
                            "AllGather", mybir.AluOpType.bypass,
                            replica_groups=[list(range(n_cores))],
                            ins=[xw_bounce1.opt()], outs=[xw_full1.opt()])

                # ---- aggregate: agg^T[:, blk] += msg^T @ S^T ----------------
                nc.vector.memset(p_sb[:], 0.0)
                agg_ps = {}
                for (tbl_ap, idx_sb, S_in, nch, sched, tsubs) in (
                        (tblA_ap, iA_sb, Sa, nchA, schedA, cfg["subsA"]),
                        (tblB_ap, iB_sb, Sb, nchB, schedB, cfg["subsB"])):
                    if skip_edges:
                        break
                    for c in range(nch):
                        msg = mpool.tile([128, subc, 128], bf16, tag="msg")
                        if skip_gather:
                            nc.vector.memset(msg[:], 0.0)
                        else:
                            nvalid = min(gchunk,
                                         max(0, tsubs * 128 - c * gchunk))
                            nc.gpsimd.dma_gather(
                                msg[:], tbl_ap,
                                idx_sb[:, c * i16s:(c + 1) * i16s],
                                num_idxs=gchunk, num_idxs_reg=nvalid,
                                elem_size=128, queue_num=0,
                                single_packet=False)
                        if skip_smm:
                            continue
                        hs = subc // 2
                        S_half = []
                        for h in range(2):
                            base = (c * subc + h * hs) * blkn
                            S_sb = stpool.tile([128, hs, blkn], bf16,
                                               name="S_sb", tag="S")
                            nc.sync.dma_start(
                                S_sb[:],
                                S_in[:, base:base + hs * blkn]
                                .rearrange("e (s m) -> e s m", m=blkn))
                            S_half.append(S_sb)
                        for (j, blk, st, sp) in sched[c]:
                            if st:
                                agg_ps[blk] = apool.tile(
                                    [d, blkn], fp32, name="aggps", tag="aggps")
                            nc.tensor.matmul(agg_ps[blk][:],
                                             lhsT=msg[:, j, 0:d],
                                             rhs=S_half[j // hs][:, j % hs, :],
                                             start=st, stop=sp)
                            if sp:
                                lo = blk * blkn
                                bw = min(blkn, npad - lo)
                                nc.vector.tensor_tensor(
                                    out=p_sb[:, lo:lo + bw],
                                    in0=p_sb[:, lo:lo + bw],
                                    in1=agg_ps.pop(blk)[:, :bw], op=Alu.add)

                # ---- epilogue (feature-major) ------------------------------
                nc.vector.tensor_scalar_add(p_sb[:], p_sb[:], bT_sb[:, i:i + 1])
                nc.vector.tensor_scalar_mul(t_sb[:], p_sb[:],
                                            prelu_sb[:d, i:i + 1])
                nc.vector.tensor_tensor(out=p_sb[:], in0=p_sb[:], in1=t_sb[:],
                                        op=Alu.max)
                if i > 0:
                    nc.vector.tensor_tensor(out=p_sb[:], in0=p_sb[:],
                                            in1=h_sb[:], op=Alu.add)
                nc.vector.tensor_tensor(out=t_sb[:], in0=p_sb[:], in1=p_sb[:],
                                        op=Alu.mult)
                nc.vector.reduce_sum(stat_sb[:, 0:1], p_sb[:, :nsh], axis=Ax.X)
                nc.vector.reduce_sum(stat_sb[:, 1:2], t_sb[:, :nsh], axis=Ax.X)
                nc.sync.dma_start(stats_in[:].rearrange("s d -> d s"),
                                  stat_sb[:])
                nc.gpsimd.collective_compute(
                    "AllReduce", mybir.AluOpType.add,
                    replica_groups=[list(range(n_cores))],
                    ins=[stats_in.opt()], outs=[stats_out.opt()])
                nc.sync.dma_start(stat2_sb[:],
                                  stats_out[:].rearrange("s d -> d s"))

                mean_c = rpool.tile([d, 1], fp32, tag="meanc")
                var_c = rpool.tile([d, 1], fp32, tag="varc")
                tmp_c = rpool.tile([d, 1], fp32, tag="tmpc")
                gs_c = rpool.tile([d, 1], fp32, tag="gsc")
                gb_c = rpool.tile([d, 1], fp32, tag="gbc")
                inv_n = 1.0 / float(n_nodes)
                nc.vector.tensor_scalar_mul(mean_c[:], stat2_sb[:, 0:1], inv_n)
                nc.vector.tensor_scalar_mul(var_c[:], stat2_sb[:, 1:2], inv_n)
                nc.vector.tensor_tensor(out=tmp_c[:], in0=mean_c[:],
                                        in1=mean_c[:], op=Alu.mult)
                nc.vector.tensor_tensor(out=var_c[:], in0=var_c[:],
                                        in1=tmp_c[:], op=Alu.subtract)
                nc.vector.tensor_scalar_add(var_c[:], var_c[:], BN_EPS)
                nc.scalar.activation(tmp_c[:], var_c[:],
                                     mybir.ActivationFunctionType.Sqrt)
                nc.vector.reciprocal(var_c[:], tmp_c[:])
                nc.vector.tensor_tensor(out=gs_c[:], in0=gaT_sb[:, i:i + 1],
                                        in1=var_c[:], op=Alu.mult)
                nc.vector.tensor_tensor(out=tmp_c[:], in0=mean_c[:],
                                        in1=gs_c[:], op=Alu.mult)
                nc.vector.tensor_tensor(out=gb_c[:], in0=beT_sb[:, i:i + 1],
                                        in1=tmp_c[:], op=Alu.subtract)
                nc.vector.tensor_scalar(out=h_sb[:], in0=p_sb[:],
                                        scalar1=gs_c[:], scalar2=gb_c[:],
                                        op0=Alu.mult, op1=Alu.add)

            for t in range(nt):
                tr_ps = ppool.tile([128, d], fp32, tag="trps")
                nc.tensor.transpose(tr_ps[:], h_sb[:, t * 128:(t + 1) * 128],
                                    ident[:])
                nc.vector.tensor_copy(out_sb[:, t, :], tr_ps[:])
            nc.sync.dma_start(out_ext[:].rearrange("(t p) d -> p t d", p=128),
                              out_sb[:])
    nc.compile()
    return nc


# ----------------------------------------------------------------------------
# Entry point
# ----------------------------------------------------------------------------

def kernel(x, edge_src, edge_dst, edge_weight, W, b, prelu_a,
           bn_gamma, bn_beta):
    from concourse.bass_utils import run_bass_kernel_spmd

    x = np.asarray(x)
    n = x.shape[0]
    nsh = n // N_CORES
    in_maps, cfg = _preprocess(x, edge_src, edge_dst, edge_weight, W, b,
                               prelu_a, bn_gamma, bn_beta,
                               N_CORES, nsh, GCHUNK, BLKN)
    nc = _build_nc(cfg)
    trace = bool(int(os.environ.get("GCN_TRACE", "0")))
    if trace:
        try:
            import antenv.axon_hooks  # noqa: F401
        except ImportError:
            trace = False
    res = run_bass_kernel_spmd(nc, in_maps, core_ids=list(range(N_CORES)),
                               trace=trace)
    LAST_RUN["results"] = res
    LAST_RUN["cfg"] = cfg
    LAST_RUN["nc"] = nc
    LAST_RUN["in_maps"] = in_maps
    out = np.concatenate(
        [res.results[r]["out"][:nsh] for r in range(N_CORES)], axis=0)
    return out.astype(np.float32)


def measure_exec_ns(nc, in_maps, n_reps=10):
    """Steady-state device-time estimate: pre-staged device inputs; marginal
    (slope) wall time of k back-to-back NEFF executions, amortizing the
    axon tunnel dispatch overhead."""
    import time
    import jax
    import concourse.mybir as mybir
    from jax.sharding import Mesh, PartitionSpec, NamedSharding
    from jax.experimental.shard_map import shard_map
    from concourse import bass2jax

    n_cores = len(in_maps)
    partition_name = (nc.partition_id_tensor.name
                      if nc.partition_id_tensor else None)
    in_names, out_names, out_avals = [], [], []
    for alloc in nc.m.functions[0].allocations:
        if not isinstance(alloc, mybir.MemoryLocationSet):
            continue
        name = alloc.memorylocations[0].name
        if alloc.kind == "ExternalInput":
            if name != partition_name:
                in_names.append(name)
        elif alloc.kind == "ExternalOutput":
            out_names.append(name)
            out_avals.append(jax.core.ShapedArray(
                tuple(alloc.tensor_shape), mybir.dt.np(alloc.dtype)))
    n_params = len(in_names)
    all_in = list(in_names) + list(out_names)
    if partition_name is not None:
        all_in.append(partition_name)

    def _body(*args):
        operands = list(args)
        if partition_name is not None:
            operands.append(bass2jax.partition_id_tensor())
        outs = bass2jax._bass_exec_p.bind(
            *operands, out_avals=tuple(out_avals), in_names=tuple(all_in),
            out_names=tuple(out_names), lowering_input_output_aliases=(),
            sim_require_finite=True, sim_require_nnan=True, nc=nc)
        return tuple(outs)

    devices = jax.devices()[:n_cores]
    mesh = Mesh(np.asarray(devices), ("core",))
    nin = n_params + len(out_names)
    fn = jax.jit(shard_map(_body, mesh=mesh,
                           in_specs=(PartitionSpec("core"),) * nin,
                           out_specs=(PartitionSpec("core"),) * len(out_names),
                           check_rep=False))
    sh = NamedSharding(mesh, PartitionSpec("core"))
    dev_in = [jax.device_put(
        np.concatenate([np.asarray(in_maps[c][k]) for c in range(n_cores)],
                       axis=0), sh) for k in in_names]
    dev_zero = [jax.device_put(
        np.zeros((n_cores * a.shape[0], *a.shape[1:]), a.dtype), sh)
        for a in out_avals]
    out = fn(*dev_in, *dev_zero)
    jax.block_until_ready(out)
    k = 16
    best1 = bestk = 1e9
    times = []
    for _ in range(n_reps):
        t0 = time.perf_counter()
        out = fn(*dev_in, *dev_zero)
        jax.block_until_ready(out)
        t = time.perf_counter() - t0
        times.append(t)
        best1 = min(best1, t)
    for _ in range(4):
        t0 = time.perf_counter()
        last = None
        for _ in range(k):
            last = fn(*dev_in, *dev_zero)
        jax.block_until_ready(last)
        bestk = min(bestk, time.perf_counter() - t0)
    marginal = (bestk - best1) / (k - 1)
    return int(marginal * 1e9), times


# revision 24
# speedup vs baseline: 1.2576x; 1.1950x over previous
"""GCN layers (3x GCNConv + PReLU + residual + BatchNorm) on 8 TRN2 NeuronCores.

Full-input contract: kernel(**inputs) takes unsharded numpy arrays and returns
the full [50000, 64] float32 output.

Sharding (graph/data parallel): nodes are partitioned into 8 contiguous
ranges; edges are bucketed to the core owning their dst node and grouped by
dst block. Per layer, per core:
  1. xw^T = W[i]^T @ h^T  (PE, feature-major), transposed+converted into a
     bf16 node-major gather table shard (padded rows of 128 cols, 64 valid)
  2. Two AllGathers build two table pieces in local HBM (the split keeps
     gather indices within int16 range, and lets table-A gathers overlap
     the second collective)
  3. stream edges: dma_gather 256B rows table[src] -> SBUF; aggregate with
     per-128-edge-subchunk segment matmuls agg^T[block] += msg^T @ S^T
     (S carries the edge weights, bf16, PSUM-accumulated per dst block)
  4. epilogue (feature-major [64, nodes]): +bias, PReLU, residual, BatchNorm
     with global batch stats via a tiny AllReduce; per-feature params are
     per-partition scalars (native tensor_scalar ops).

The per-block subchunk schedule is made uniform across cores (max-padded)
so all cores run the same SPMD program.
"""

import os
import numpy as np

N_NODES = 50000
D = 64
L = 3
BN_EPS = 1e-5
N_CORES = 8
GCHUNK = 6144           # edge slots per gather chunk (48 subchunks of 128)
BLKN = 512              # dst nodes per aggregation block (S columns)
IDX_LIMIT = 32768       # int16 gather index range

LAST_RUN = {}


# ----------------------------------------------------------------------------
# Host-side preprocessing
# ----------------------------------------------------------------------------

def _wrap16(flat, slots):
    """Edge-slot array -> [128, slots/16] int16 'wrapped' index layout."""
    a = flat.reshape(slots // 16, 16).T.astype(np.int16)
    return np.tile(a, (8, 1))


def _preprocess(x, edge_src, edge_dst, edge_weight, W, b, prelu_a,
                bn_gamma, bn_beta, n_cores, nsh, gchunk, blkn,
                split_ag=True):
    import ml_dtypes
    bf16 = ml_dtypes.bfloat16

    n = x.shape[0]
    d = x.shape[1]
    nt = (nsh + 127) // 128
    npad = nt * 128
    subc = gchunk // 128
    nblk = (npad + blkn - 1) // blkn
    # per-rank local-row split: table A = local rows [0, asplit), B = rest
    if npad * n_cores <= IDX_LIMIT:
        asplit = max(128, (npad // 2) // 128 * 128)
    else:
        asplit = (IDX_LIMIT // n_cores) // 128 * 128

    src = np.asarray(edge_src).astype(np.int64)
    dst = np.asarray(edge_dst).astype(np.int64)
    w = np.asarray(edge_weight).astype(np.float32)
    x = np.asarray(x).astype(np.float32)

    s_rank = src // nsh
    s_loc = src % nsh
    if split_ag:
        inA = s_loc < asplit
        idxA = s_rank * asplit + s_loc
        idxB = s_rank * (npad - asplit) + (s_loc - asplit)
    else:
        src_pad = s_rank * npad + s_loc
        gsplit = min(IDX_LIMIT, ((npad * n_cores) // 2) // 128 * 128)
        inA = src_pad < gsplit
        idxA = src_pad
        idxB = src_pad - gsplit
        asplit = gsplit  # reused as the global row split below
    shard = dst // nsh
    dst_local = dst % nsh

    streams = []
    for sel, tix in ((inA, idxA), (~inA, idxB)):
        per_core_edges = []
        cnts = []
        for r in range(n_cores):
            m = (shard == r) & sel
            per_core_edges.append((tix[m], dst_local[m], w[m]))
            cnts.append(np.bincount(dst_local[m] // blkn, minlength=nblk))
        nsub = np.zeros(nblk, np.int64)
        for c in cnts:
            nsub = np.maximum(nsub, (c + 127) // 128)
        sub_off = np.concatenate([[0], np.cumsum(nsub)])
        total_subs = int(sub_off[-1])
        nch = max(1, (total_subs + subc - 1) // subc)
        padded_subs = nch * subc
        slots = padded_subs * 128

        sched = []
        for blk in range(nblk):
            for j in range(int(nsub[blk])):
                gsub = int(sub_off[blk]) + j
                sched.append((gsub // subc, gsub % subc, blk,
                              j == 0, j == int(nsub[blk]) - 1))

        per_core = []
        for r in range(n_cores):
            ti, dl, wr = per_core_edges[r]
            blk = dl // blkn
            col = dl % blkn
            order = np.argsort(blk, kind="stable")
            ti, wr, blk, col = (a[order] for a in (ti, wr, blk, col))
            cnt = cnts[r]
            starts = np.concatenate([[0], np.cumsum(cnt)])
            pos = np.arange(len(ti)) - starts[blk]
            gsub = sub_off[blk] + pos // 128
            row = pos % 128
            slot = gsub * 128 + row
            rng_pad = np.random.default_rng(12345 + r)
            trows_s = int(ti.max()) + 1 if len(ti) else 1
            idx = rng_pad.integers(0, trows_s, slots)
            idx[total_subs * 128:] = -1   # uniform tail dummies: no descriptors
            idx[slot] = ti
            S = np.zeros((padded_subs, 128, blkn), np.float32)
            S[gsub, row, col] = wr
            St = np.ascontiguousarray(S.transpose(1, 0, 2)
                                      .reshape(128, padded_subs * blkn))
            per_core.append((_wrap16(idx, slots), St.astype(bf16)))
        streams.append(dict(nch=nch, slots=slots, sched=sched,
                            total_subs=total_subs, per_core=per_core))

    bT = np.ascontiguousarray(np.asarray(b, np.float32).T)
    gammaT = np.ascontiguousarray(np.asarray(bn_gamma, np.float32).T)
    betaT = np.ascontiguousarray(np.asarray(bn_beta, np.float32).T)
    prelu_rep = np.tile(np.asarray(prelu_a, np.float32).reshape(1, L),
                        (128, 1))
    Wf = np.ascontiguousarray(np.asarray(W, np.float32))

    in_maps = []
    for r in range(n_cores):
        h0T = np.zeros((d, npad), np.float32)
        h0T[:, :nsh] = x[r * nsh:(r + 1) * nsh].T
        in_maps.append({
            "h0sT": h0T,
            "Wp": Wf,
            "bT": bT,
            "gammaT": gammaT,
            "betaT": betaT,
            "prelu_rep": prelu_rep,
            "srcA": streams[0]["per_core"][r][0],
            "Sa": streams[0]["per_core"][r][1],
            "srcB": streams[1]["per_core"][r][0],
            "Sb": streams[1]["per_core"][r][1],
        })

    cfg = dict(n_cores=n_cores, nsh=nsh, d=d, nt=nt, npad=npad,
               gchunk=gchunk, subc=subc, blkn=blkn, nblk=nblk,
               asplit=asplit, split_ag=split_ag, n_nodes=n,
               nchA=streams[0]["nch"], slotsA=streams[0]["slots"],
               schedA=streams[0]["sched"], subsA=streams[0]["total_subs"],
               nchB=streams[1]["nch"], slotsB=streams[1]["slots"],
               schedB=streams[1]["sched"], subsB=streams[1]["total_subs"])
    return in_maps, cfg


# ----------------------------------------------------------------------------
# Device program
# ----------------------------------------------------------------------------

def _build_nc(cfg):
    import concourse.bacc as bacc
    import concourse.tile as tile
    import concourse.mybir as mybir
    from concourse import library_config
    from concourse.masks import make_identity

    fp32 = mybir.dt.float32
    bf16 = mybir.dt.bfloat16
    i16 = mybir.dt.int16
    Alu = mybir.AluOpType
    Ax = mybir.AxisListType

    n_cores = cfg["n_cores"]
    nsh, d, nt, npad = cfg["nsh"], cfg["d"], cfg["nt"], cfg["npad"]
    gchunk, subc = cfg["gchunk"], cfg["subc"]
    blkn, nblk, asplit = cfg["blkn"], cfg["nblk"], cfg["asplit"]
    n_nodes = cfg["n_nodes"]
    slotsA, slotsB = cfg["slotsA"], cfg["slotsB"]
    nchA, nchB = cfg["nchA"], cfg["nchB"]
    split_ag = cfg.get("split_ag", True)
    if split_ag:
        rows_a = n_cores * asplit
        rows_b = n_cores * (npad - asplit)
    else:
        rows_a = asplit
        rows_b = n_cores * npad - asplit
    i16s = gchunk // 16
    nxc = (npad + 511) // 512

    def by_chunk(sched, nch):
        per = [[] for _ in range(nch)]
        for (c, j, blk, st, sp) in sched:
            per[c].append((j, blk, st, sp))
        return per

    schedA = by_chunk(cfg["schedA"], nchA)
    schedB = by_chunk(cfg["schedB"], nchB)
    skip_gather = cfg.get("skip_gather", False)
    skip_smm = cfg.get("skip_smm", False)
    skip_edges = cfg.get("skip_edges", False)
    skip_ag = cfg.get("skip_ag", False)

    nc = bacc.Bacc(None, target_bir_lowering=False, debug=False)

    h0sT = nc.declare_dram_parameter("h0sT", [d, npad], fp32, isOutput=False)
    Wp = nc.declare_dram_parameter("Wp", [L, d, d], fp32, isOutput=False)
    bT_in = nc.declare_dram_parameter("bT", [d, L], fp32, isOutput=False)
    gammaT_in = nc.declare_dram_parameter("gammaT", [d, L], fp32, isOutput=False)
    betaT_in = nc.declare_dram_parameter("betaT", [d, L], fp32, isOutput=False)
    prelu_in = nc.declare_dram_parameter("prelu_rep", [128, L], fp32, isOutput=False)
    srcA = nc.declare_dram_parameter("srcA", [128, slotsA // 16], i16, isOutput=False)
    Sa = nc.declare_dram_parameter("Sa", [128, slotsA * blkn // 128], bf16, isOutput=False)
    srcB = nc.declare_dram_parameter("srcB", [128, slotsB // 16], i16, isOutput=False)
    Sb = nc.declare_dram_parameter("Sb", [128, slotsB * blkn // 128], bf16, isOutput=False)
    out_ext = nc.declare_dram_parameter("out", [npad, d], fp32, isOutput=True)

    with tile.TileContext(nc) as tc:
        with (
            tc.tile_pool(name="const", bufs=1) as cpool,
            tc.tile_pool(name="state", bufs=1) as spool,
            tc.tile_pool(name="meta", bufs=1) as epool,
            tc.tile_pool(name="work", bufs=2) as wpool,
            tc.tile_pool(name="rows", bufs=2) as rpool,
            tc.tile_pool(name="msg", bufs=2) as mpool,
            tc.tile_pool(name="smat", bufs=2) as stpool,
            tc.tile_pool(name="ps", bufs=2, space="PSUM") as ppool,
            tc.tile_pool(name="psagg", bufs=2, space="PSUM") as apool,
            tc.tile_pool(name="dram", bufs=1, space="DRAM") as dpool,
        ):
            h_sb = spool.tile([d, npad], fp32, tag="h")
            p_sb = spool.tile([d, npad], fp32, tag="p")
            t_sb = spool.tile([d, npad], fp32, tag="t")
            xwbf = spool.tile([128, nt, 128], bf16, tag="xwbf")
            out_sb = spool.tile([128, nt, d], fp32, tag="osb")
            ident = cpool.tile([d, d], fp32, tag="ident")
            W_sb = cpool.tile([d, L * d], fp32, tag="Wsb")
            bT_sb = cpool.tile([d, L], fp32, tag="bT")
            gaT_sb = cpool.tile([d, L], fp32, tag="gaT")
            beT_sb = cpool.tile([d, L], fp32, tag="beT")
            prelu_sb = cpool.tile([128, L], fp32, tag="prelu")
            stat_sb = spool.tile([d, 2], fp32, tag="stat")
            stat2_sb = spool.tile([d, 2], fp32, tag="stat2")
            iA_sb = epool.tile([128, slotsA // 16], i16, tag="iA")
            iB_sb = epool.tile([128, slotsB // 16], i16, tag="iB")

            if split_ag:
                xw_bounceA = dpool.tile([asplit, 128], bf16, tag="xwbA")
                xw_bounceB = dpool.tile([npad - asplit, 128], bf16, tag="xwbB")
                xw_fullA = dpool.tile([rows_a, 128], bf16, tag="xwfA")
                xw_fullB = dpool.tile([rows_b, 128], bf16, tag="xwfB")
                tblA_ap, tblB_ap = xw_fullA[:], xw_fullB[:]
            else:
                xw_bounce1 = dpool.tile([npad, 128], bf16, tag="xwb1")
                xw_full1 = dpool.tile([n_cores * npad, 128], bf16, tag="xwf1")
                tblA_ap = xw_full1[0:rows_a, :]
                tblB_ap = xw_full1[rows_a:n_cores * npad, :]
            stats_in = dpool.tile([2, d], fp32, tag="sin")
            stats_out = dpool.tile([2, d], fp32, tag="sout")

            nc.sync.dma_start(h_sb[:], h0sT[:])
            for i in range(L):
                nc.sync.dma_start(W_sb[:, i * d:(i + 1) * d], Wp[i, :, :])
            nc.sync.dma_start(bT_sb[:], bT_in[:])
            nc.sync.dma_start(gaT_sb[:], gammaT_in[:])
            nc.sync.dma_start(beT_sb[:], betaT_in[:])
            nc.sync.dma_start(prelu_sb[:], prelu_in[:])
            nc.sync.dma_start(iA_sb[:], srcA[:])
            nc.sync.dma_start(iB_sb[:], srcB[:])
            make_identity(nc, ident[:])
            nc.vector.memset(xwbf[:], 0.0)
            nc.gpsimd.load_library(library_config.mlp)

            for i in range(L):
                # ---- xw^T = W[i]^T @ h^T, transpose+convert -----------------
                for c in range(nxc):
                    lo = c * 512
                    hi = min(npad, lo + 512)
                    xw_ps = ppool.tile([d, 512], fp32, tag="xwps")
                    nc.tensor.matmul(xw_ps[:, :hi - lo],
                                     lhsT=W_sb[:, i * d:(i + 1) * d],
                                     rhs=h_sb[:, lo:hi], start=True, stop=True)
                    xwT_tmp = wpool.tile([d, 512], fp32, tag="xwt")
                    nc.vector.tensor_copy(xwT_tmp[:, :hi - lo],
                                          xw_ps[:, :hi - lo])
                    for k in range((hi - lo) // 128):
                        t = (lo + k * 128) // 128
                        tr_ps = ppool.tile([128, d], fp32, tag="trps")
                        nc.tensor.transpose(
                            tr_ps[:], xwT_tmp[:, k * 128:(k + 1) * 128],
                            ident[:])
                        nc.vector.tensor_copy(xwbf[:, t, 0:d], tr_ps[:])

                # ---- AllGather the table ------------------------------------
                if split_ag:
                    ta = asplit // 128
                    nc.sync.dma_start(
                        xw_bounceA[:].rearrange("(t p) m -> p t m", p=128),
                        xwbf[:, 0:ta, :])
                    nc.sync.dma_start(
                        xw_bounceB[:].rearrange("(t p) m -> p t m", p=128),
                        xwbf[:, ta:nt, :])
                    if skip_ag:
                        nc.sync.dma_start(xw_fullA[0:asplit, :],
                                          xw_bounceA[:])
                        nc.sync.dma_start(xw_fullB[0:npad - asplit, :],
                                          xw_bounceB[:])
                    else:
                        nc.gpsimd.collective_compute(
                            "AllGather", mybir.AluOpType.bypass,
                            replica_groups=[list(range(n_cores))],
                            ins=[xw_bounceA.opt()], outs=[xw_fullA.opt()])
                        nc.gpsimd.collective_compute(
                            "AllGather", mybir.AluOpType.bypass,
                            replica_groups=[list(range(n_cores))],
                            ins=[xw_bounceB.opt()], outs=[xw_fullB.opt()])
                else:
                    nc.sync.dma_start(
                        xw_bounce1[:].rearrange("(t p) m -> p t m", p=128),
                        xwbf[:])
                    if skip_ag:
                        nc.sync.dma_start(xw_full1[0:npad, :], xw_bounce1[:])
                    else:
                        nc.gpsimd.collective_compute(
                            "AllGather", mybir.AluOpType.bypass,
                            replica_groups=[list(range(n_cores))],
                            ins=[xw_bounce1.opt()], outs=[xw_full1.opt()])

                # ---- aggregate: agg^T[:, blk] += msg^T @ S^T ----------------
                nc.vector.memset(p_sb[:], 0.0)
                agg_ps = {}
                for (tbl_ap, idx_sb, S_in, nch, sched, tsubs) in (
                        (tblA_ap, iA_sb, Sa, nchA, schedA, cfg["subsA"]),
                        (tblB_ap, iB_sb, Sb, nchB, schedB, cfg["subsB"])):
                    if skip_edges:
                        break
                    for c in range(nch):
                        msg = mpool.tile([128, subc, 128], bf16, tag="msg")
                        if skip_gather:
                            nc.vector.memset(msg[:], 0.0)
                        else:
                            nvalid = min(gchunk,
                                         max(0, tsubs * 128 - c * gchunk))
                            nc.gpsimd.dma_gather(
                                msg[:], tbl_ap,
                                idx_sb[:, c * i16s:(c + 1) * i16s],
                                num_idxs=gchunk, num_idxs_reg=nvalid,
                                elem_size=128, queue_num=0,
                                single_packet=False)
                        if skip_smm:
                            continue
                        hs = subc // 2
                        S_half = []
                        for h in range(2):
                            base = (c * subc + h * hs) * blkn
                            S_sb = stpool.tile([128, hs, blkn], bf16,
                                               name="S_sb", tag="S")
                            nc.sync.dma_start(
                                S_sb[:],
                                S_in[:, base:base + hs * blkn]
                                .rearrange("e (s m) -> e s m", m=blkn))
                            S_half.append(S_sb)
                        for (j, blk, st, sp) in sched[c]:
                            if st:
                                agg_ps[blk] = apool.tile(
                                    [d, blkn], fp32, name="aggps", tag="aggps")
                            nc.tensor.matmul(agg_ps[blk][:],
                                             lhsT=msg[:, j, 0:d],
                                             rhs=S_half[j // hs][:, j % hs, :],
                                             start=st, stop=sp)
                            if sp:
                                lo = blk * blkn
                                bw = min(blkn, npad - lo)
                                nc.vector.tensor_tensor(
                                    out=p_sb[:, lo:lo + bw],
                                    in0=p_sb[:, lo:lo + bw],
                                    in1=agg_ps.pop(blk)[:, :bw], op=Alu.add)

                # ---- epilogue (feature-major) ------------------------------
                nc.vector.tensor_scalar_add(p_sb[:], p_sb[:], bT_sb[:, i:i + 1])
                nc.vector.tensor_scalar_mul(t_sb[:], p_sb[:],
                                            prelu_sb[:d, i:i + 1])
                nc.vector.tensor_tensor(out=p_sb[:], in0=p_sb[:], in1=t_sb[:],
                                        op=Alu.max)
                if i > 0:
                    nc.vector.tensor_tensor(out=p_sb[:], in0=p_sb[:],
                                            in1=h_sb[:], op=Alu.add)
                nc.vector.tensor_tensor(out=t_sb[:], in0=p_sb[:], in1=p_sb[:],
                                        op=Alu.mult)
                nc.vector.reduce_sum(stat_sb[:, 0:1], p_sb[:, :nsh], axis=Ax.X)
                nc.vector.reduce_sum(stat_sb[:, 1:2], t_sb[:, :nsh], axis=Ax.X)
                nc.sync.dma_start(stats_in[:].rearrange("s d -> d s"),
                                  stat_sb[:])
                nc.gpsimd.collective_compute(
                    "AllReduce", mybir.AluOpType.add,
                    replica_groups=[list(range(n_cores))],
                    ins=[stats_in.opt()], outs=[stats_out.opt()])
                nc.sync.dma_start(stat2_sb[:],
                                  stats_out[:].rearrange("s d -> d s"))

                mean_c = rpool.tile([d, 1], fp32, tag="meanc")
                var_c = rpool.tile([d, 1], fp32, tag="varc")
                tmp_c = rpool.tile([d, 1], fp32, tag="tmpc")
                gs_c = rpool.tile([d, 1], fp32, tag="gsc")
                gb_c = rpool.tile([d, 1], fp32, tag="gbc")
                inv_n = 1.0 / float(n_nodes)
                nc.vector.tensor_scalar_mul(mean_c[:], stat2_sb[:, 0:1], inv_n)
                nc.vector.tensor_scalar_mul(var_c[:], stat2_sb[:, 1:2], inv_n)
                nc.vector.tensor_tensor(out=tmp_c[:], in0=mean_c[:],
                                        in1=mean_c[:], op=Alu.mult)
                nc.vector.tensor_tensor(out=var_c[:], in0=var_c[:],
                                        in1=tmp_c[:], op=Alu.subtract)
                nc.vector.tensor_scalar_add(var_c[:], var_c[:], BN_EPS)
                nc.scalar.activation(tmp_c[:], var_c[:],
                                     mybir.ActivationFunctionType.Sqrt)
                nc.vector.reciprocal(var_c[:], tmp_c[:])
                nc.vector.tensor_tensor(out=gs_c[:], in0=gaT_sb[:, i:i + 1],
                                        in1=var_c[:], op=Alu.mult)
                nc.vector.tensor_tensor(out=tmp_c[:], in0=mean_c[:],
                                        in1=gs_c[:], op=Alu.mult)
                nc.vector.tensor_tensor(out=gb_c[:], in0=beT_sb[:, i:i + 1],
                                        in1=tmp_c[:], op=Alu.subtract)
                nc.vector.tensor_scalar(out=h_sb[:], in0=p_sb[:],
                                        scalar1=gs_c[:], scalar2=gb_c[:],
                                        op0=Alu.mult, op1=Alu.add)

            for t in range(nt):
                tr_ps = ppool.tile([128, d], fp32, tag="trps")
                nc.tensor.transpose(tr_ps[:], h_sb[:, t * 128:(t + 1) * 128],
                                    ident[:])
                nc.vector.tensor_copy(out_sb[:, t, :], tr_ps[:])
            nc.sync.dma_start(out_ext[:].rearrange("(t p) d -> p t d", p=128),
                              out_sb[:])
    nc.compile()
    return nc


# ----------------------------------------------------------------------------
# Entry point
# ----------------------------------------------------------------------------

def kernel(x, edge_src, edge_dst, edge_weight, W, b, prelu_a,
           bn_gamma, bn_beta):
    from concourse.bass_utils import run_bass_kernel_spmd

    x = np.asarray(x)
    n = x.shape[0]
    nsh = n // N_CORES
    in_maps, cfg = _preprocess(x, edge_src, edge_dst, edge_weight, W, b,
                               prelu_a, bn_gamma, bn_beta,
                               N_CORES, nsh, GCHUNK, BLKN)
    nc = _build_nc(cfg)
    trace = bool(int(os.environ.get("GCN_TRACE", "0")))
    if trace:
        try:
            import antenv.axon_hooks  # noqa: F401
        except ImportError:
            trace = False
    res = run_bass_kernel_spmd(nc, in_maps, core_ids=list(range(N_CORES)),
                               trace=trace)
    LAST_RUN["results"] = res
    LAST_RUN["cfg"] = cfg
    LAST_RUN["nc"] = nc
    LAST_RUN["in_maps"] = in_maps
    out = np.concatenate(
        [res.results[r]["out"][:nsh] for r in range(N_CORES)], axis=0)
    return out.astype(np.float32)


def measure_exec_ns(nc, in_maps, n_reps=10):
    """Steady-state device-time estimate: pre-staged device inputs; marginal
    (slope) wall time of k back-to-back NEFF executions, amortizing the
    axon tunnel dispatch overhead."""
    import time
    import jax
    import concourse.mybir as mybir
    from jax.sharding import Mesh, PartitionSpec, NamedSharding
    from jax.experimental.shard_map import shard_map
    from concourse import bass2jax

    n_cores = len(in_maps)
    partition_name = (nc.partition_id_tensor.name
                      if nc.partition_id_tensor else None)
    in_names, out_names, out_avals = [], [], []
    for alloc in nc.m.functions[0].allocations:
        if not isinstance(alloc, mybir.MemoryLocationSet):
            continue
        name = alloc.memorylocations[0].name
        if alloc.kind == "ExternalInput":
            if name != partition_name:
                in_names.append(name)
        elif alloc.kind == "ExternalOutput":
            out_names.append(name)
            out_avals.append(jax.core.ShapedArray(
                tuple(alloc.tensor_shape), mybir.dt.np(alloc.dtype)))
    n_params = len(in_names)
    all_in = list(in_names) + list(out_names)
    if partition_name is not None:
        all_in.append(partition_name)

    def _body(*args):
        operands = list(args)
        if partition_name is not None:
            operands.append(bass2jax.partition_id_tensor())
        outs = bass2jax._bass_exec_p.bind(
            *operands, out_avals=tuple(out_avals), in_names=tuple(all_in),
            out_names=tuple(out_names), lowering_input_output_aliases=(),
            sim_require_finite=True, sim_require_nnan=True, nc=nc)
        return tuple(outs)

    devices = jax.devices()[:n_cores]
    mesh = Mesh(np.asarray(devices), ("core",))
    nin = n_params + len(out_names)
    fn = jax.jit(shard_map(_body, mesh=mesh,
                           in_specs=(PartitionSpec("core"),) * nin,
                           out_specs=(PartitionSpec("core"),) * len(out_names),
                           check_rep=False))
    sh = NamedSharding(mesh, PartitionSpec("core"))
    dev_in = [jax.device_put(
        np.concatenate([np.asarray(in_maps[c][k]) for c in range(n_cores)],
                       axis=0), sh) for k in in_names]
    dev_zero = [jax.device_put(
        np.zeros((n_cores * a.shape[0], *a.shape[1:]), a.dtype), sh)
        for a in out_avals]
    out = fn(*dev_in, *dev_zero)
    jax.block_until_ready(out)

    def best_of(k, reps):
        best = 1e9
        for _ in range(reps):
            t0 = time.perf_counter()
            last = None
            for _ in range(k):
                last = fn(*dev_in, *dev_zero)
            jax.block_until_ready(last)
            best = min(best, time.perf_counter() - t0)
        return best

    # slope between two amortized queue depths; robust to the flaky
    # single-call dispatch floor of the axon tunnel
    t8 = best_of(8, 5)
    t32 = best_of(32, 5)
    marginal = (t32 - t8) / 24
    times = [t8, t32]
    return int(marginal * 1e9), times
